# revision 55
# baseline (speedup 1.0000x reference)
"""Trainium2 Bass kernel for DiT focused-linear-attention block (nn_DiT_9259949490457).

Data-parallel over batch: 16 batches -> 8 NeuronCores, 2 batches/core, no collectives.
Host pre-transposes x -> xT (and y back), so the PE does only essential GEMM
columns: q-GEMM, kv-GEMM, einsum1/2, proj (feature-major, bias as per-partition
ACT bias), plus the depthwise 3x3 conv as clipped diagonal matmuls (center tap
first so psum pending-zero state stays uniform). Focus-norm row sums run as DVE
mul+reduce pairs (TensorTensorReduce breaks on HW); acc2 alternates ACT/DVE for
balance. Per-head q3 tiles come from a DRAM roundtrip (contiguous-row reads,
SWDGE-issued); q3 spills are ACT-issued so the SP/HWDGE queue stays clear for
x chunks. einsum2 evacuates via ACT copy + DVE piece-adds into 128-row-aligned
OT chunk tiles consumed by proj. Batch-1's dy=0 conv taps run on DVE (which has
slack during proj-b0), trimming ~12k diagonal-matmul columns off the PE.
"""

import numpy as np
import ml_dtypes

import concourse.bacc as bacc
import concourse.mybir as mybir
import concourse.tile as tile
from concourse import bass_utils

F32 = mybir.dt.float32
BF16 = mybir.dt.bfloat16
ALU = mybir.AluOpType
AF = mybir.ActivationFunctionType
AX = mybir.AxisListType

NCORES = 8
B, N, DIM = 16, 1024, 1152
H, KVH, HD = 12, 4, 96
BL = B // NCORES          # 2 local batches
T = BL * N                # 2048 local tokens
NK = DIM // 128           # 9 feature K-tiles
TT = N // 128             # 8 token tiles per batch
C4 = T // 512             # 4 free-dim chunks of 512 over all local tokens
TAPS = [(dy, dx) for dy in (-1, 0, 1) for dx in (-1, 0, 1)]

_BF = ml_dtypes.bfloat16


def _spanp(b):
    if b % 128 == 0:
        return 128
    if b % 64 == 0:
        return 64
    return 32


def _head_pieces(h):
    """Split head h's 96 feature rows into pieces legal for partition-offset
    access both at the 128-aligned global row (r0) and the within-head row (rr).
    Returns [(j_tile, r0, rr, cnt)]."""
    out = []
    rr = 0
    while rr < 96:
        gr = 96 * h + rr
        j, r0 = divmod(gr, 128)
        cnt = min(96 - rr, 128 - r0, _spanp(r0), _spanp(rr))
        out.append((j, r0, rr, cnt))
        rr += cnt
    return out


def _build_kernel():
    nc = bacc.Bacc("TRN2", target_bir_lowering=False, debug=False,
                   enable_asserts=True, num_devices=NCORES)
    xT_in = nc.dram_tensor("xT", [DIM, T], BF16, kind="ExternalInput").ap()
    wqT_in = nc.dram_tensor("wqT", [DIM, DIM], BF16, kind="ExternalInput").ap()
    wkvT_in = nc.dram_tensor("wkvT", [DIM, 768], BF16, kind="ExternalInput").ap()
    pwT_in = nc.dram_tensor("pwT", [DIM, DIM], BF16, kind="ExternalInput").ap()
    wqb_in = nc.dram_tensor("wqb", [128, NK], F32, kind="ExternalInput").ap()
    kvb_in = nc.dram_tensor("kvb", [1, 384], BF16, kind="ExternalInput").ap()
    kvbbc_in = nc.dram_tensor("kvbbc", [128, 384], BF16, kind="ExternalInput").ap()
    pjb_in = nc.dram_tensor("pjb", [128, NK], F32, kind="ExternalInput").ap()
    dwcw_in = nc.dram_tensor("dwcw", [96, KVH, 9], F32, kind="ExternalInput").ap()
    dwcb_in = nc.dram_tensor("dwcb", [96, KVH], F32, kind="ExternalInput").ap()
    masks_in = nc.dram_tensor("masks", [128, NK, H], BF16, kind="ExternalInput").ap()
    diagp_in = nc.dram_tensor("diagp", [96, KVH, 9, 96], BF16, kind="ExternalInput").ap()
    y_out = nc.dram_tensor("y", [DIM, T], F32, kind="ExternalOutput").ap()

    from contextlib import ExitStack
    with tile.TileContext(nc) as tc, ExitStack() as stack:
        cpool = stack.enter_context(tc.tile_pool(name="const", bufs=1))
        dpool = stack.enter_context(tc.tile_pool(name="dram", bufs=1, space="DRAM"))
        wp = stack.enter_context(tc.tile_pool(name="work", bufs=1))
        pmm = stack.enter_context(tc.tile_pool(name="pmm", bufs=1, space="PSUM"))
        pa = stack.enter_context(tc.tile_pool(name="pa", bufs=2, space="PSUM"))

        # ---- consts (Pool/SWDGE path, parallel with HWDGE x loads below) ----
        WqT = [cpool.tile([128, DIM], BF16, name=f"WqT{k}") for k in range(NK)]
        WkvT = [cpool.tile([128, 768], BF16, name=f"WkvT{k}") for k in range(NK)]
        PWT = [cpool.tile([128, DIM], BF16, name=f"PWT{k}") for k in range(NK)]
        wqb = cpool.tile([128, NK], F32, name="wqb")
        kvb = cpool.tile([1, 384], BF16, name="kvb")
        kvbbc = cpool.tile([128, 384], BF16, name="kvbbc")
        pjb = cpool.tile([128, NK], F32, name="pjb")
        dwcw = cpool.tile([96, KVH, 9], F32, name="dwcw")
        dwcb = cpool.tile([96, KVH], F32, name="dwcb")
        masks = cpool.tile([128, NK, H], BF16, name="masks")
        diagP = cpool.tile([96, KVH, 9, 96], BF16, name="diagP")
        ones_r = cpool.tile([1, 128], BF16, name="ones_r")
        ones_c = cpool.tile([128, 1], BF16, name="ones_c")
        nc.vector.memset(ones_r[:], 1.0)
        nc.vector.memset(ones_c[:], 1.0)

        xT = [cpool.tile([128, T], BF16, name=f"xT{k}") for k in range(NK)]
        # SP/HWDGE: interleave x chunk-0 with WqT (both needed immediately),
        # then x c4=1, then WkvT (scheduler hoists K work into G1), then the
        # rest of x. Pool/SWDGE: wqb first, then late-needed consts.
        for half in range(2):
            for k in range(NK):
                nc.sync.dma_start(
                    out=xT[k][:, 1024 * half:1024 * (half + 1)],
                    in_=xT_in[128 * k:128 * (k + 1), 1024 * half:1024 * (half + 1)])
        for k in range(NK):
            nc.gpsimd.dma_start(out=WqT[k][:], in_=wqT_in[128 * k:128 * (k + 1), :])
        nc.gpsimd.dma_start(out=wqb[:], in_=wqb_in[:])
        for k in range(NK):
            nc.gpsimd.dma_start(out=WkvT[k][:], in_=wkvT_in[128 * k:128 * (k + 1), :])
        nc.gpsimd.dma_start(out=kvb[:], in_=kvb_in[:])
        nc.gpsimd.dma_start(out=kvbbc[:], in_=kvbbc_in[:])
        nc.gpsimd.dma_start(out=masks[:], in_=masks_in[:])
        nc.gpsimd.dma_start(out=diagP[:], in_=diagp_in[:])
        nc.gpsimd.dma_start(out=dwcw[:], in_=dwcw_in[:])
        nc.gpsimd.dma_start(out=dwcb[:], in_=dwcb_in[:])
        for k in range(NK):
            nc.gpsimd.dma_start(out=PWT[k][:], in_=pwT_in[128 * k:128 * (k + 1), :])
        nc.gpsimd.dma_start(out=pjb[:], in_=pjb_in[:])

        vpad = dpool.tile([BL, N, KVH, 128], BF16, name="vpad")
        q3d = dpool.tile([BL, DIM, N], BF16, name="q3d")

        # accs: col = (j, c4) for q, (g, t) for k
        acc1q = wp.tile([128, NK, C4], F32, name="acc1q", tag="acc1q")
        acc2q = wp.tile([128, NK, C4], F32, name="acc2q", tag="acc2q")
        acc1k = wp.tile([128, KVH, 2 * TT], F32, name="acc1k", tag="acc1k")
        acc2k = wp.tile([128, KVH, 2 * TT], F32, name="acc2k", tag="acc2k")

        q3h = [[wp.tile([96, N], BF16, name=f"q3h{b}_{h}", tag=f"q3h_{h}", bufs=1)
                for h in range(H)] for b in range(BL)]

        # ---------------- phase G1: q GEMM + focus(q) ----------------
        def g1_evac_fn(j, c4, pqj):
            u = wp.tile([128, 512], BF16, name="u", tag="u", bufs=2)
            nc.scalar.activation(u[:], pqj[:], AF.Relu, bias=wqb[:, j:j + 1])
            u2 = wp.tile([128, 512], BF16, name="u2", tag="u2", bufs=2)
            nc.vector.tensor_mul(u2[:], u[:], u[:])
            nc.vector.tensor_reduce(out=acc1q[:, j, c4:c4 + 1], in_=u2[:],
                                    axis=AX.X, op=ALU.add)
            q3s = wp.tile([128, 512], BF16, name="q3s", tag="q3s", bufs=3)
            nc.vector.tensor_mul(q3s[:], u2[:], u[:])
            junk = wp.tile([128, 512], BF16, name="junk", tag="junk", bufs=2)
            if (c4 + j) % 2 == 0:
                nc.scalar.activation(junk[:], q3s[:], AF.Square,
                                     accum_out=acc2q[:, j, c4:c4 + 1])
            else:
                nc.vector.tensor_mul(junk[:], q3s[:], q3s[:])
                nc.vector.tensor_reduce(out=acc2q[:, j, c4:c4 + 1],
                                        in_=junk[:], axis=AX.X, op=ALU.add)
            nc.scalar.dma_start(
                out=q3d[c4 // 2, 128 * j:128 * (j + 1),
                        512 * (c4 % 2):512 * (c4 % 2 + 1)],
                in_=q3s[:])

        # c4=0 is paced by x-tile arrival: run j0-7 k-outer (8 psums, ~1.7us
        # of PE work per arriving tile), j8 mops up once x is resident.
        p8 = {}
        for j in range(8):
            if j < 4:
                p8[j] = pmm.tile([128, 512], F32, name=f"pq{j}", tag=f"pq{j}")
            elif j < 7:
                p8[j] = pa.tile([128, 512], F32, name="pqx", tag="pe2", bufs=3)
            else:
                p8[j] = pa.tile([128, 512], F32, name="pqy", tag="pa", bufs=1)
        for k in range(NK):
            for j in range(8):
                nc.tensor.matmul(p8[j][:], WqT[k][:, 128 * j:128 * (j + 1)],
                                 xT[k][:, 0:512],
                                 start=(k == 0), stop=(k == NK - 1))
        for j in range(8):
            g1_evac_fn(j, 0, p8[j])
        pq8 = pmm.tile([128, 512], F32, name="pq0", tag="pq0")
        for k in range(NK):
            nc.tensor.matmul(pq8[:], WqT[k][:, 128 * 8:DIM],
                             xT[k][:, 0:512], start=(k == 0), stop=(k == NK - 1))
        g1_evac_fn(8, 0, pq8)

        for c4 in range(1, C4):
            t0 = 512 * c4
            for jg in ((0, 1, 2), (3, 4, 5), (6, 7, 8)):
                def _g1psum(j):
                    m = j % 5
                    if m < 4:
                        return pmm.tile([128, 512], F32, name=f"pq{m}",
                                        tag=f"pq{m}")
                    return pa.tile([128, 512], F32, name="pqx",
                                   tag="pe2", bufs=3)
                pq = {j: _g1psum(j) for j in jg}
                for k in range(NK):
                    for j in jg:
                        nc.tensor.matmul(pq[j][:], WqT[k][:, 128 * j:128 * (j + 1)],
                                         xT[k][:, t0:t0 + 512],
                                         start=(k == 0), stop=(k == NK - 1))
                for j in jg:
                    g1_evac_fn(j, c4, pq[j])
            if c4 % 2 == 1:
                # batch c4//2's q3d fully written: fetch per-head tiles now so
                # they are resident long before einsum2 needs them.
                bq = c4 // 2
                for h in range(H):
                    nc.gpsimd.dma_start(out=q3h[bq][h][:],
                                        in_=q3d[bq, 96 * h:96 * (h + 1), :])

        # ---------------- phase K/V + per-batch tail ----------------
        k3 = [wp.tile([128, 384], BF16, name=f"k3_{t}", tag=f"k3_{t}")
              for t in range(2 * TT)]
        vv = [wp.tile([128, 384], BF16, name=f"v_{t}", tag=f"v_{t}")
              for t in range(2 * TT)]
        kvp = [[wp.tile([96, 96], BF16, name=f"kvp{b}_{h}", tag=f"kvp_{h}", bufs=2)
                for h in range(H)] for b in range(BL)]
        vd = [[wp.tile([96, N], BF16, name=f"vd{b}_{g}", tag=f"vd_{g}", bufs=1)
               for g in range(KVH)] for b in range(BL)]
        gbs = []

        vTt = [[wp.tile([128, N], BF16, name=f"vT{b}_{g}", tag=f"vT_{g}", bufs=1)
                for g in range(KVH)] for b in range(BL)]

        def emit_vtrans(b):
            for g in range(KVH):
                nc.sync.dma_start(out=vTt[b][g][:], in_=vpad[b, :, g, :],
                                  transpose=True)

        def dwc_compute(b, dysel=(-1, 0, 1)):
            # all 9 taps as clipped diagonal matmuls accumulating in psum
            # halves; dwcb bias folded into the ACT evacuation.
            for g in range(KVH):
                v3 = vTt[b][g][:].rearrange("p (y x) -> p y x", y=32)
                pd = [pa.tile([96, 512], F32, name=f"pd{hf}", tag="pe2", bufs=3)
                      for hf in range(2)]
                p3 = [pd[hf][:].rearrange("p (y x) -> p y x", y=16)
                      for hf in range(2)]
                emitted = [False, False]
                last = [None, None]
                plan = []
                # center tap first per half: full coverage under start=True so
                # psum pending-zero state stays uniform for later partial taps
                taps_sorted = sorted(enumerate(TAPS),
                                     key=lambda e: (e[1] != (0, 0) and
                                                    e[1] != (-1, 0),))
                for ti, (dy, dx) in taps_sorted:
                    if dy not in dysel:
                        continue
                    x0, x1 = max(0, -dx), 32 - max(0, dx)
                    for hf in (0, 1):
                        y0 = max(16 * hf, -dy if dy < 0 else 0)
                        y1 = min(16 * hf + 16, 32 - max(0, dy))
                        if y1 > y0:
                            plan.append((ti, dy, dx, hf, y0, y1, x0, x1))
                            last[hf] = len(plan) - 1
                for pi, (ti, dy, dx, hf, y0, y1, x0, x1) in enumerate(plan):
                    nc.tensor.matmul(
                        p3[hf][:, y0 - 16 * hf:y1 - 16 * hf, x0:x1],
                        diagP[:, g, ti, :],
                        v3[0:96, y0 + dy:y1 + dy, x0 + dx:x1 + dx],
                        start=not emitted[hf], stop=(pi == last[hf]))
                    emitted[hf] = True
                for hf in range(2):
                    nc.scalar.activation(vd[b][g][:, 512 * hf:512 * (hf + 1)],
                                         pd[hf][:], AF.Identity,
                                         bias=dwcb[:, g:g + 1])

        for b in range(BL):
          with tc.tile_wait_until(0.030, enable=False):
            for t in range(TT * b, TT * (b + 1)):
                t0 = 128 * t
                pk = pmm.tile([128, 512], F32, name="pk", tag=f"pq{t % 4}")
                for k in range(NK):
                    nc.tensor.matmul(pk[:, 0:384], xT[k][:, t0:t0 + 128],
                                     WkvT[k][:, 0:384],
                                     start=(k == 0), stop=False)
                nc.tensor.matmul(pk[:, 0:384], ones_r[:], kvb[:],
                                 start=False, stop=True)
                uk = wp.tile([128, 384], BF16, name="uk", tag="uk", bufs=2)
                nc.scalar.activation(uk[:], pk[:, 0:384], AF.Relu)
                k2 = wp.tile([128, 384], BF16, name="k2", tag="k2", bufs=2)
                nc.vector.tensor_mul(k2[:], uk[:], uk[:])
                nc.vector.tensor_reduce(
                    out=acc1k[:, :, t], in_=k2[:].rearrange("p (g d) -> p g d", g=KVH),
                    axis=AX.X, op=ALU.add)
                nc.vector.tensor_mul(k3[t][:], k2[:], uk[:])
                junkk = wp.tile([128, 384], BF16, name="junkk", tag="junk", bufs=2)
                nc.vector.tensor_mul(junkk[:], k3[t][:], k3[t][:])
                nc.vector.tensor_reduce(
                    out=acc2k[:, :, t], in_=junkk[:].rearrange("p (g d) -> p g d", g=KVH),
                    axis=AX.X, op=ALU.add)
            for t in range(TT * b, TT * (b + 1)):
                t0 = 128 * t
                pv = pmm.tile([128, 512], F32, name="pv", tag=f"pq{t % 4}")
                for k in range(NK):
                    nc.tensor.matmul(pv[:, 0:384], xT[k][:, t0:t0 + 128],
                                     WkvT[k][:, 384:768],
                                     start=(k == 0), stop=(k == NK - 1))
                nc.vector.tensor_tensor(out=vv[t][:], in0=pv[:, 0:384],
                                        in1=kvbbc[:], op=ALU.add)
                nc.sync.dma_start(
                    out=vpad[b, 128 * (t - TT * b):128 * (t - TT * b + 1), :, 0:96],
                    in_=vv[t][:].rearrange("p (k d) -> p k d", k=KVH))
            emit_vtrans(b)
            if b == 1:
                dwc_compute(0)

            # ---- norms -> per-head scale gb (tiny) ----
            sq_rows = []
            for acc in (acc1q, acc2q):
                accs = wp.tile([128, NK], F32, name="accs", tag="accs", bufs=1)
                nc.vector.tensor_add(accs[:], acc[:, :, 2 * b], acc[:, :, 2 * b + 1])
                accsb = wp.tile([128, NK], BF16, name="accsb", tag="accsb", bufs=1)
                nc.vector.tensor_copy(accsb[:], accs[:])
                psn = pa.tile([1, H], F32, name="psn", tag="pa", bufs=1)
                for j in range(NK):
                    nc.tensor.matmul(psn[:], accsb[:, j:j + 1], masks[:, j, :],
                                     start=(j == 0), stop=(j == NK - 1))
                srow = wp.tile([1, H], F32, name="srow", tag="srow", bufs=2)
                nc.vector.tensor_copy(srow[:], psn[:])
                sq_rows.append(srow)
            sk_rows = []
            for acc in (acc1k, acc2k):
                accb = wp.tile([128, KVH * TT], BF16, name="accb", tag="accb", bufs=1)
                nc.vector.tensor_copy(accb[:], acc[:, :, TT * b:TT * (b + 1)])
                psk = pa.tile([1, KVH * TT], F32, name="psk", tag="pa", bufs=1)
                nc.tensor.matmul(psk[:], ones_c[:], accb[:], start=True, stop=True)
                krow = wp.tile([1, KVH * TT], F32, name="krow", tag="krow", bufs=1)
                nc.vector.tensor_copy(krow[:], psk[:])
                kred = wp.tile([1, KVH], F32, name="kred", tag="kred", bufs=2)
                nc.vector.tensor_reduce(kred[:],
                                        krow[:].rearrange("a (k t) -> a k t", k=KVH),
                                        axis=AX.X, op=ALU.add)
                sk_rows.append(kred)

            def _f_row(s1, s2, width, tagp):
                se = wp.tile([1, width], F32, name="se", tag=f"se{tagp}", bufs=1)
                nc.vector.tensor_scalar_add(se[:], s2[:], 1e-30)
                rc = wp.tile([1, width], F32, name="rc", tag=f"rc{tagp}", bufs=1)
                nc.vector.reciprocal(rc[:], se[:])
                rt = wp.tile([1, width], F32, name="rt", tag=f"rt{tagp}", bufs=1)
                nc.vector.tensor_mul(rt[:], s1[:], rc[:])
                fr = wp.tile([1, width], F32, name="fr", tag=f"fr{tagp}", bufs=1)
                nc.scalar.activation(fr[:], rt[:], AF.Sqrt)
                return fr

            fq = _f_row(sq_rows[0], sq_rows[1], H, "q")
            fk = _f_row(sk_rows[0], sk_rows[1], KVH, "k")
            fk12 = wp.tile([1, H], F32, name="fk12", tag="fk12", bufs=1)
            for g in range(3):
                nc.vector.tensor_copy(fk12[:, 4 * g:4 * (g + 1)], fk[:])
            grow = wp.tile([1, H], F32, name="grow", tag="grow", bufs=1)
            nc.vector.tensor_mul(grow[:], fq[:], fk12[:])
            gb = wp.tile([96, H], F32, name="gb", tag="gb", bufs=2)
            nc.gpsimd.partition_broadcast(gb[:], grow[:], channels=96)
            gbs.append(gb)

            # ---- einsum1 + scale ----
            for g in range(KVH):
                pk_t = pa.tile([96, 96], F32, name="pkvt", tag="pa", bufs=1)
                for i, t in enumerate(range(TT * b, TT * (b + 1))):
                    nc.tensor.matmul(pk_t[:], k3[t][:, 96 * g:96 * (g + 1)],
                                     vv[t][:, 96 * g:96 * (g + 1)],
                                     start=(i == 0), stop=(i == TT - 1))
                for h in range(g, H, KVH):
                    nc.vector.tensor_scalar(out=kvp[b][h][:], in0=pk_t[:],
                                            scalar1=gb[:, h:h + 1], scalar2=None,
                                            op0=ALU.mult)


        # ---------------- einsum2 + combine -> OTc, proj ----------------
        OTc = [[wp.tile([128, 512], BF16, name=f"OT_{j}_{c}", tag=f"OT_{j}_{c}",
                        bufs=1) for c in range(2)] for j in range(NK)]

        def emit_e2(b, c2, h):
            g = h % KVH
            pe2 = pa.tile([96, 512], F32, name="pe2", tag="pe2", bufs=3)
            nc.tensor.matmul(pe2[:], kvp[b][h][:],
                             q3h[b][h][:, 512 * c2:512 * (c2 + 1)],
                             start=True, stop=True)
            pac = wp.tile([96, 512], BF16, name="pac", tag="pac", bufs=2)
            nc.scalar.copy(pac[:], pe2[:])
            for (j, r0, rr, cnt) in _head_pieces(h):
                nc.vector.tensor_tensor(
                    out=OTc[j][c2][r0:r0 + cnt, :],
                    in0=pac[rr:rr + cnt, :],
                    in1=vd[b][g][rr:rr + cnt, 512 * c2:512 * (c2 + 1)],
                    op=ALU.add)

        def emit_proj(b, c2, jo):
            py = pmm.tile([128, 512], F32, name="py", tag=f"pq{jo % 4}")
            for k in range(NK):
                nc.tensor.matmul(py[:], PWT[k][:, 128 * jo:128 * (jo + 1)],
                                 OTc[k][c2][:], start=(k == 0), stop=(k == NK - 1))
            ysb = wp.tile([128, 512], F32, name="ysb", tag="ysb", bufs=2)
            nc.scalar.activation(ysb[:], py[:], AF.Identity, bias=pjb[:, jo:jo + 1])
            t0 = 1024 * b + 512 * c2
            nc.sync.dma_start(out=y_out[128 * jo:128 * (jo + 1), t0:t0 + 512],
                              in_=ysb[:])

        # b0 einsum2 (both chunks), then proj b0 c2=0;
        # e2 b1 c2=0 interleaves into proj b0 c2=1 (OTc rings free as proj b0
        # finishes reading each chunk), e2 b1 c2=1 into proj b1 c2=0.
        for c2 in range(2):
            for h in range(H):
                emit_e2(0, c2, h)
        # batch-1 dwconv split: PE keeps the dy=+-1 taps (psum + ACT bias
        # evac), DVE adds the dy=0 row (fits its slack during proj b0).
        dwc_compute(1, dysel=(-1, 1))
        for g in range(KVH):
            v3 = vTt[1][g][:].rearrange("p (y x) -> p y x", y=32)
            o3 = vd[1][g][:].rearrange("p (y x) -> p y x", y=32)
            for ti, (dy, dx) in enumerate(TAPS):
                if dy != 0:
                    continue
                y0, y1 = 0, 32
                x0, x1 = max(0, -dx), 32 - max(0, dx)
                tmp = wp.tile([96, N], BF16, name="tmp", tag="dtmp", bufs=1)
                t3 = tmp[:].rearrange("p (y x) -> p y x", y=32)
                nc.vector.tensor_scalar(
                    out=t3[:, y0:y1, x0:x1],
                    in0=v3[0:96, y0 + dy:y1 + dy, x0 + dx:x1 + dx],
                    scalar1=dwcw[:, g, ti:ti + 1], scalar2=None, op0=ALU.mult)
                nc.vector.tensor_tensor(
                    out=o3[:, y0:y1, x0:x1], in0=o3[:, y0:y1, x0:x1],
                    in1=t3[:, y0:y1, x0:x1], op=ALU.add)
        for jo in range(NK):
            emit_proj(0, 0, jo)
        e2q = [(1, 0, h) for h in range(H)]
        for jo in range(NK):
            emit_proj(0, 1, jo)
            for _ in range(2):
                if e2q:
                    emit_e2(*e2q.pop(0))
        while e2q:
            emit_e2(*e2q.pop(0))
        e2q = [(1, 1, h) for h in range(H)]
        for jo in range(NK):
            emit_proj(1, 0, jo)
            for _ in range(2):
                if e2q:
                    emit_e2(*e2q.pop(0))
        while e2q:
            emit_e2(*e2q.pop(0))
        for jo in range(NK):
            emit_proj(1, 1, jo)

    nc.compile()
    return nc


_NC_CACHE = None


def _get_nc():
    global _NC_CACHE
    if _NC_CACHE is None:
        _NC_CACHE = _build_kernel()
    return _NC_CACHE


def _host_consts(wq_w, wq_b, wkv_w, wkv_b, dwc_w, dwc_b, proj_w, proj_b):
    wqT = np.ascontiguousarray(np.asarray(wq_w, np.float32).T).astype(_BF)
    wkvT = np.ascontiguousarray(np.asarray(wkv_w, np.float32).T).astype(_BF)
    pwT = np.ascontiguousarray(np.asarray(proj_w, np.float32).T).astype(_BF)
    wqb = np.ascontiguousarray(np.asarray(wq_b, np.float32).reshape(NK, 128).T)
    kvb_full = np.asarray(wkv_b, np.float32).reshape(1, 768)
    kvb_r = kvb_full[:, 0:384].astype(_BF)
    kvbbc = np.broadcast_to(kvb_full[:, 384:768], (128, 384)).astype(_BF)
    pjb = np.ascontiguousarray(np.asarray(proj_b, np.float32).reshape(NK, 128).T)
    dw = np.asarray(dwc_w, np.float32).reshape(KVH, 96, 9)
    dwcw = np.ascontiguousarray(dw.transpose(1, 0, 2))
    dwcb = np.ascontiguousarray(np.asarray(dwc_b, np.float32).reshape(KVH, 96).T)
    mk = np.zeros((128, NK, H), np.float32)
    for j in range(NK):
        for p in range(128):
            f = 128 * j + p
            mk[p, j, f // 96] = 1.0
    masks = mk.astype(_BF)
    dgv = np.zeros((96, KVH, 9, 96), np.float32)
    for d in range(96):
        dgv[d, :, :, d] = dw[:, d, :]
    diagp = dgv.astype(_BF)
    return dict(wqT=wqT, wkvT=wkvT, pwT=pwT, wqb=wqb, kvb=kvb_r, kvbbc=kvbbc,
                pjb=pjb, dwcw=dwcw, dwcb=dwcb, masks=masks, diagp=diagp)


def kernel(x, wq_w, wq_b, wkv_w, wkv_b, dwc_w, dwc_b, proj_w, proj_b,
           _want_results=False, **_unused):
    nc = _get_nc()
    consts = _host_consts(wq_w, wq_b, wkv_w, wkv_b, dwc_w, dwc_b, proj_w, proj_b)
    x = np.asarray(x, np.float32)
    in_maps = []
    for c in range(NCORES):
        m = dict(consts)
        m["xT"] = np.ascontiguousarray(
            x[BL * c:BL * (c + 1)].reshape(T, DIM).T).astype(_BF)
        in_maps.append(m)
    res = bass_utils.run_bass_kernel_spmd(nc, in_maps, core_ids=list(range(NCORES)))
    y = np.stack([np.ascontiguousarray(res.results[c]["y"].T).reshape(BL, N, DIM)
                  for c in range(NCORES)])
    y = y.reshape(B, N, DIM)
    if _want_results:
        return y, res
    return y


# revision 56
# speedup vs baseline: 1.0165x; 1.0165x over previous
"""Trainium2 Bass kernel for DiT focused-linear-attention block (nn_DiT_9259949490457).

Data-parallel over batch: 16 batches -> 8 NeuronCores, 2 batches/core, no collectives.
Host pre-transposes x -> xT (and y back), so the PE does only essential GEMM
columns: q-GEMM, kv-GEMM, einsum1/2, proj (feature-major, bias as per-partition
ACT bias), plus the depthwise 3x3 conv as clipped diagonal matmuls (center tap
first so psum pending-zero state stays uniform). Focus-norm row sums run as DVE
mul+reduce pairs (TensorTensorReduce breaks on HW); acc2 alternates ACT/DVE for
balance. Per-head q3 tiles come from a DRAM roundtrip (contiguous-row reads,
SWDGE-issued); q3 spills are ACT-issued so the SP/HWDGE queue stays clear for
x chunks. einsum2 evacuates via ACT copy + DVE piece-adds into 128-row-aligned
OT chunk tiles consumed by proj.
"""

import numpy as np
import ml_dtypes

import concourse.bacc as bacc
import concourse.mybir as mybir
import concourse.tile as tile
from concourse import bass_utils

F32 = mybir.dt.float32
BF16 = mybir.dt.bfloat16
ALU = mybir.AluOpType
AF = mybir.ActivationFunctionType
AX = mybir.AxisListType

NCORES = 8
B, N, DIM = 16, 1024, 1152
H, KVH, HD = 12, 4, 96
BL = B // NCORES          # 2 local batches
T = BL * N                # 2048 local tokens
NK = DIM // 128           # 9 feature K-tiles
TT = N // 128             # 8 token tiles per batch
C4 = T // 512             # 4 free-dim chunks of 512 over all local tokens
TAPS = [(dy, dx) for dy in (-1, 0, 1) for dx in (-1, 0, 1)]

_BF = ml_dtypes.bfloat16


def _spanp(b):
    if b % 128 == 0:
        return 128
    if b % 64 == 0:
        return 64
    return 32


def _head_pieces(h):
    """Split head h's 96 feature rows into pieces legal for partition-offset
    access both at the 128-aligned global row (r0) and the within-head row (rr).
    Returns [(j_tile, r0, rr, cnt)]."""
    out = []
    rr = 0
    while rr < 96:
        gr = 96 * h + rr
        j, r0 = divmod(gr, 128)
        cnt = min(96 - rr, 128 - r0, _spanp(r0), _spanp(rr))
        out.append((j, r0, rr, cnt))
        rr += cnt
    return out


def _build_kernel():
    nc = bacc.Bacc("TRN2", target_bir_lowering=False, debug=False,
                   enable_asserts=True, num_devices=NCORES)
    xT_in = nc.dram_tensor("xT", [DIM, T], BF16, kind="ExternalInput").ap()
    wqT_in = nc.dram_tensor("wqT", [DIM, DIM], BF16, kind="ExternalInput").ap()
    wkvT_in = nc.dram_tensor("wkvT", [DIM, 768], BF16, kind="ExternalInput").ap()
    pwT_in = nc.dram_tensor("pwT", [DIM, DIM], BF16, kind="ExternalInput").ap()
    wqb_in = nc.dram_tensor("wqb", [128, NK], F32, kind="ExternalInput").ap()
    kvb_in = nc.dram_tensor("kvb", [1, 384], BF16, kind="ExternalInput").ap()
    kvbbc_in = nc.dram_tensor("kvbbc", [128, 384], BF16, kind="ExternalInput").ap()
    pjb_in = nc.dram_tensor("pjb", [128, NK], F32, kind="ExternalInput").ap()
    dwcw_in = nc.dram_tensor("dwcw", [96, KVH, 9], F32, kind="ExternalInput").ap()
    dwcb_in = nc.dram_tensor("dwcb", [96, KVH], F32, kind="ExternalInput").ap()
    masks_in = nc.dram_tensor("masks", [128, NK, H], BF16, kind="ExternalInput").ap()
    diagp_in = nc.dram_tensor("diagp", [96, KVH, 9, 96], BF16, kind="ExternalInput").ap()
    y_out = nc.dram_tensor("y", [DIM, T], F32, kind="ExternalOutput").ap()

    from contextlib import ExitStack
    with tile.TileContext(nc) as tc, ExitStack() as stack:
        cpool = stack.enter_context(tc.tile_pool(name="const", bufs=1))
        dpool = stack.enter_context(tc.tile_pool(name="dram", bufs=1, space="DRAM"))
        wp = stack.enter_context(tc.tile_pool(name="work", bufs=1))
        pmm = stack.enter_context(tc.tile_pool(name="pmm", bufs=1, space="PSUM"))
        pa = stack.enter_context(tc.tile_pool(name="pa", bufs=2, space="PSUM"))

        # ---- consts (Pool/SWDGE path, parallel with HWDGE x loads below) ----
        WqT = [cpool.tile([128, DIM], BF16, name=f"WqT{k}") for k in range(NK)]
        WkvT = [cpool.tile([128, 768], BF16, name=f"WkvT{k}") for k in range(NK)]
        PWT = [cpool.tile([128, DIM], BF16, name=f"PWT{k}") for k in range(NK)]
        wqb = cpool.tile([128, NK], F32, name="wqb")
        kvb = cpool.tile([1, 384], BF16, name="kvb")
        kvbbc = cpool.tile([128, 384], BF16, name="kvbbc")
        pjb = cpool.tile([128, NK], F32, name="pjb")
        dwcw = cpool.tile([96, KVH, 9], F32, name="dwcw")
        dwcb = cpool.tile([96, KVH], F32, name="dwcb")
        masks = cpool.tile([128, NK, H], BF16, name="masks")
        diagP = cpool.tile([96, KVH, 9, 96], BF16, name="diagP")
        ones_r = cpool.tile([1, 128], BF16, name="ones_r")
        ones_c = cpool.tile([128, 1], BF16, name="ones_c")
        nc.vector.memset(ones_r[:], 1.0)
        nc.vector.memset(ones_c[:], 1.0)

        xT = [cpool.tile([128, T], BF16, name=f"xT{k}") for k in range(NK)]
        # SP/HWDGE: interleave x chunk-0 with WqT (both needed immediately),
        # then x c4=1, then WkvT (scheduler hoists K work into G1), then the
        # rest of x. Pool/SWDGE: wqb first, then late-needed consts.
        for half in range(2):
            for k in range(NK):
                nc.sync.dma_start(
                    out=xT[k][:, 1024 * half:1024 * (half + 1)],
                    in_=xT_in[128 * k:128 * (k + 1), 1024 * half:1024 * (half + 1)])
        for k in range(NK):
            nc.gpsimd.dma_start(out=WqT[k][:], in_=wqT_in[128 * k:128 * (k + 1), :])
        nc.gpsimd.dma_start(out=wqb[:], in_=wqb_in[:])
        for k in range(NK):
            nc.gpsimd.dma_start(out=WkvT[k][:], in_=wkvT_in[128 * k:128 * (k + 1), :])
        nc.gpsimd.dma_start(out=kvb[:], in_=kvb_in[:])
        nc.gpsimd.dma_start(out=kvbbc[:], in_=kvbbc_in[:])
        nc.gpsimd.dma_start(out=masks[:], in_=masks_in[:])
        nc.gpsimd.dma_start(out=diagP[:], in_=diagp_in[:])
        nc.gpsimd.dma_start(out=dwcw[:], in_=dwcw_in[:])
        nc.gpsimd.dma_start(out=dwcb[:], in_=dwcb_in[:])
        for k in range(NK):
            nc.gpsimd.dma_start(out=PWT[k][:], in_=pwT_in[128 * k:128 * (k + 1), :])
        nc.gpsimd.dma_start(out=pjb[:], in_=pjb_in[:])

        vpad = dpool.tile([BL, N, KVH, 128], BF16, name="vpad")
        q3d = dpool.tile([BL, DIM, N], BF16, name="q3d")

        # accs: col = (j, c4) for q, (g, t) for k
        acc1q = wp.tile([128, NK, C4], F32, name="acc1q", tag="acc1q")
        acc2q = wp.tile([128, NK, C4], F32, name="acc2q", tag="acc2q")
        acc1k = wp.tile([128, KVH, 2 * TT], F32, name="acc1k", tag="acc1k")
        acc2k = wp.tile([128, KVH, 2 * TT], F32, name="acc2k", tag="acc2k")

        q3h = [[wp.tile([96, N], BF16, name=f"q3h{b}_{h}", tag=f"q3h_{h}", bufs=1)
                for h in range(H)] for b in range(BL)]

        # ---------------- phase G1: q GEMM + focus(q) ----------------
        for c4 in range(C4):
            t0 = 512 * c4
            for jg in ((0, 1, 2), (3, 4, 5), (6, 7, 8)):
                def _g1psum(j):
                    m = j % 5
                    if m < 4:
                        return pmm.tile([128, 512], F32, name=f"pq{m}",
                                        tag=f"pq{m}")
                    return pa.tile([128, 512], F32, name="pqx",
                                   tag="pe2", bufs=3)
                pq = {j: _g1psum(j) for j in jg}
                for k in range(NK):
                    for j in jg:
                        nc.tensor.matmul(pq[j][:], WqT[k][:, 128 * j:128 * (j + 1)],
                                         xT[k][:, t0:t0 + 512],
                                         start=(k == 0), stop=(k == NK - 1))
                for j in jg:
                    u = wp.tile([128, 512], BF16, name="u", tag="u", bufs=2)
                    nc.scalar.activation(u[:], pq[j][:], AF.Relu, bias=wqb[:, j:j + 1])
                    u2 = wp.tile([128, 512], BF16, name="u2", tag="u2", bufs=2)
                    nc.vector.tensor_mul(u2[:], u[:], u[:])
                    nc.vector.tensor_reduce(out=acc1q[:, j, c4:c4 + 1], in_=u2[:],
                                            axis=AX.X, op=ALU.add)
                    q3s = wp.tile([128, 512], BF16, name="q3s", tag="q3s", bufs=3)
                    nc.vector.tensor_mul(q3s[:], u2[:], u[:])
                    junk = wp.tile([128, 512], BF16, name="junk", tag="junk",
                                   bufs=2)
                    if (c4 + j) % 2 == 0:
                        nc.scalar.activation(junk[:], q3s[:], AF.Square,
                                             accum_out=acc2q[:, j, c4:c4 + 1])
                    else:
                        nc.vector.tensor_mul(junk[:], q3s[:], q3s[:])
                        nc.vector.tensor_reduce(out=acc2q[:, j, c4:c4 + 1],
                                                in_=junk[:], axis=AX.X, op=ALU.add)
                    b = c4 // 2
                    nc.scalar.dma_start(
                        out=q3d[b, 128 * j:128 * (j + 1),
                                512 * (c4 % 2):512 * (c4 % 2 + 1)],
                        in_=q3s[:])
            if c4 % 2 == 1:
                # batch c4//2's q3d fully written: fetch per-head tiles now so
                # they are resident long before einsum2 needs them.
                bq = c4 // 2
                for h in range(H):
                    nc.gpsimd.dma_start(out=q3h[bq][h][:],
                                        in_=q3d[bq, 96 * h:96 * (h + 1), :])

        # ---------------- phase K/V + per-batch tail ----------------
        k3 = [wp.tile([128, 384], BF16, name=f"k3_{t}", tag=f"k3_{t}")
              for t in range(2 * TT)]
        vv = [wp.tile([128, 384], BF16, name=f"v_{t}", tag=f"v_{t}")
              for t in range(2 * TT)]
        kvp = [[wp.tile([96, 96], BF16, name=f"kvp{b}_{h}", tag=f"kvp_{h}", bufs=2)
                for h in range(H)] for b in range(BL)]
        vd = [[wp.tile([96, N], BF16, name=f"vd{b}_{g}", tag=f"vd_{g}", bufs=1)
               for g in range(KVH)] for b in range(BL)]
        gbs = []

        vTt = [[wp.tile([128, N], BF16, name=f"vT{b}_{g}", tag=f"vT_{g}", bufs=1)
                for g in range(KVH)] for b in range(BL)]

        def emit_vtrans(b):
            for g in range(KVH):
                nc.sync.dma_start(out=vTt[b][g][:], in_=vpad[b, :, g, :],
                                  transpose=True)

        def dwc_compute(b, dysel=(-1, 0, 1)):
            # all 9 taps as clipped diagonal matmuls accumulating in psum
            # halves; dwcb bias folded into the ACT evacuation.
            for g in range(KVH):
                v3 = vTt[b][g][:].rearrange("p (y x) -> p y x", y=32)
                pd = [pa.tile([96, 512], F32, name=f"pd{hf}", tag="pe2", bufs=3)
                      for hf in range(2)]
                p3 = [pd[hf][:].rearrange("p (y x) -> p y x", y=16)
                      for hf in range(2)]
                emitted = [False, False]
                last = [None, None]
                plan = []
                # center tap first per half: full coverage under start=True so
                # psum pending-zero state stays uniform for later partial taps
                taps_sorted = sorted(enumerate(TAPS),
                                     key=lambda e: (e[1] != (0, 0) and
                                                    e[1] != (-1, 0),))
                for ti, (dy, dx) in taps_sorted:
                    if dy not in dysel:
                        continue
                    x0, x1 = max(0, -dx), 32 - max(0, dx)
                    for hf in (0, 1):
                        y0 = max(16 * hf, -dy if dy < 0 else 0)
                        y1 = min(16 * hf + 16, 32 - max(0, dy))
                        if y1 > y0:
                            plan.append((ti, dy, dx, hf, y0, y1, x0, x1))
                            last[hf] = len(plan) - 1
                for pi, (ti, dy, dx, hf, y0, y1, x0, x1) in enumerate(plan):
                    nc.tensor.matmul(
                        p3[hf][:, y0 - 16 * hf:y1 - 16 * hf, x0:x1],
                        diagP[:, g, ti, :],
                        v3[0:96, y0 + dy:y1 + dy, x0 + dx:x1 + dx],
                        start=not emitted[hf], stop=(pi == last[hf]))
                    emitted[hf] = True
                for hf in range(2):
                    nc.scalar.activation(vd[b][g][:, 512 * hf:512 * (hf + 1)],
                                         pd[hf][:], AF.Identity,
                                         bias=dwcb[:, g:g + 1])

        for b in range(BL):
          # keep K off the scheduler's early-hoist list until WkvT has landed
          with tc.tile_wait_until(0.030, enable=(b == 0)):
            for t in range(TT * b, TT * (b + 1)):
                t0 = 128 * t
                pk = pmm.tile([128, 512], F32, name="pk", tag=f"pq{t % 4}")
                for k in range(NK):
                    nc.tensor.matmul(pk[:, 0:384], xT[k][:, t0:t0 + 128],
                                     WkvT[k][:, 0:384],
                                     start=(k == 0), stop=False)
                nc.tensor.matmul(pk[:, 0:384], ones_r[:], kvb[:],
                                 start=False, stop=True)
                uk = wp.tile([128, 384], BF16, name="uk", tag="uk", bufs=2)
                nc.scalar.activation(uk[:], pk[:, 0:384], AF.Relu)
                k2 = wp.tile([128, 384], BF16, name="k2", tag="k2", bufs=2)
                nc.vector.tensor_mul(k2[:], uk[:], uk[:])
                nc.vector.tensor_reduce(
                    out=acc1k[:, :, t], in_=k2[:].rearrange("p (g d) -> p g d", g=KVH),
                    axis=AX.X, op=ALU.add)
                nc.vector.tensor_mul(k3[t][:], k2[:], uk[:])
                junkk = wp.tile([128, 384], BF16, name="junkk", tag="junk", bufs=2)
                nc.vector.tensor_mul(junkk[:], k3[t][:], k3[t][:])
                nc.vector.tensor_reduce(
                    out=acc2k[:, :, t], in_=junkk[:].rearrange("p (g d) -> p g d", g=KVH),
                    axis=AX.X, op=ALU.add)
            for t in range(TT * b, TT * (b + 1)):
                t0 = 128 * t
                pv = pmm.tile([128, 512], F32, name="pv", tag=f"pq{t % 4}")
                for k in range(NK):
                    nc.tensor.matmul(pv[:, 0:384], xT[k][:, t0:t0 + 128],
                                     WkvT[k][:, 384:768],
                                     start=(k == 0), stop=(k == NK - 1))
                nc.vector.tensor_tensor(out=vv[t][:], in0=pv[:, 0:384],
                                        in1=kvbbc[:], op=ALU.add)
                nc.sync.dma_start(
                    out=vpad[b, 128 * (t - TT * b):128 * (t - TT * b + 1), :, 0:96],
                    in_=vv[t][:].rearrange("p (k d) -> p k d", k=KVH))
            emit_vtrans(b)
            if b == 1:
                dwc_compute(0)

            # ---- norms -> per-head scale gb (tiny) ----
            sq_rows = []
            for acc in (acc1q, acc2q):
                accs = wp.tile([128, NK], F32, name="accs", tag="accs", bufs=1)
                nc.vector.tensor_add(accs[:], acc[:, :, 2 * b], acc[:, :, 2 * b + 1])
                accsb = wp.tile([128, NK], BF16, name="accsb", tag="accsb", bufs=1)
                nc.vector.tensor_copy(accsb[:], accs[:])
                psn = pa.tile([1, H], F32, name="psn", tag="pa", bufs=1)
                for j in range(NK):
                    nc.tensor.matmul(psn[:], accsb[:, j:j + 1], masks[:, j, :],
                                     start=(j == 0), stop=(j == NK - 1))
                srow = wp.tile([1, H], F32, name="srow", tag="srow", bufs=2)
                nc.vector.tensor_copy(srow[:], psn[:])
                sq_rows.append(srow)
            sk_rows = []
            for acc in (acc1k, acc2k):
                accb = wp.tile([128, KVH * TT], BF16, name="accb", tag="accb", bufs=1)
                nc.vector.tensor_copy(accb[:], acc[:, :, TT * b:TT * (b + 1)])
                psk = pa.tile([1, KVH * TT], F32, name="psk", tag="pa", bufs=1)
                nc.tensor.matmul(psk[:], ones_c[:], accb[:], start=True, stop=True)
                krow = wp.tile([1, KVH * TT], F32, name="krow", tag="krow", bufs=1)
                nc.vector.tensor_copy(krow[:], psk[:])
                kred = wp.tile([1, KVH], F32, name="kred", tag="kred", bufs=2)
                nc.vector.tensor_reduce(kred[:],
                                        krow[:].rearrange("a (k t) -> a k t", k=KVH),
                                        axis=AX.X, op=ALU.add)
                sk_rows.append(kred)

            def _f_row(s1, s2, width, tagp):
                se = wp.tile([1, width], F32, name="se", tag=f"se{tagp}", bufs=1)
                nc.vector.tensor_scalar_add(se[:], s2[:], 1e-30)
                rc = wp.tile([1, width], F32, name="rc", tag=f"rc{tagp}", bufs=1)
                nc.vector.reciprocal(rc[:], se[:])
                rt = wp.tile([1, width], F32, name="rt", tag=f"rt{tagp}", bufs=1)
                nc.vector.tensor_mul(rt[:], s1[:], rc[:])
                fr = wp.tile([1, width], F32, name="fr", tag=f"fr{tagp}", bufs=1)
                nc.scalar.activation(fr[:], rt[:], AF.Sqrt)
                return fr

            fq = _f_row(sq_rows[0], sq_rows[1], H, "q")
            fk = _f_row(sk_rows[0], sk_rows[1], KVH, "k")
            fk12 = wp.tile([1, H], F32, name="fk12", tag="fk12", bufs=1)
            for g in range(3):
                nc.vector.tensor_copy(fk12[:, 4 * g:4 * (g + 1)], fk[:])
            grow = wp.tile([1, H], F32, name="grow", tag="grow", bufs=1)
            nc.vector.tensor_mul(grow[:], fq[:], fk12[:])
            gb = wp.tile([96, H], F32, name="gb", tag="gb", bufs=2)
            nc.gpsimd.partition_broadcast(gb[:], grow[:], channels=96)
            gbs.append(gb)

            # ---- einsum1 + scale ----
            for g in range(KVH):
                pk_t = pa.tile([96, 96], F32, name="pkvt", tag="pa", bufs=1)
                for i, t in enumerate(range(TT * b, TT * (b + 1))):
                    nc.tensor.matmul(pk_t[:], k3[t][:, 96 * g:96 * (g + 1)],
                                     vv[t][:, 96 * g:96 * (g + 1)],
                                     start=(i == 0), stop=(i == TT - 1))
                for h in range(g, H, KVH):
                    nc.vector.tensor_scalar(out=kvp[b][h][:], in0=pk_t[:],
                                            scalar1=gb[:, h:h + 1], scalar2=None,
                                            op0=ALU.mult)


        # ---------------- einsum2 + combine -> OTc, proj ----------------
        OTc = [[wp.tile([128, 512], BF16, name=f"OT_{j}_{c}", tag=f"OT_{j}_{c}",
                        bufs=1) for c in range(2)] for j in range(NK)]

        def emit_e2(b, c2, h):
            g = h % KVH
            pe2 = pa.tile([96, 512], F32, name="pe2", tag="pe2", bufs=3)
            nc.tensor.matmul(pe2[:], kvp[b][h][:],
                             q3h[b][h][:, 512 * c2:512 * (c2 + 1)],
                             start=True, stop=True)
            pac = wp.tile([96, 512], BF16, name="pac", tag="pac", bufs=2)
            nc.scalar.copy(pac[:], pe2[:])
            for (j, r0, rr, cnt) in _head_pieces(h):
                nc.vector.tensor_tensor(
                    out=OTc[j][c2][r0:r0 + cnt, :],
                    in0=pac[rr:rr + cnt, :],
                    in1=vd[b][g][rr:rr + cnt, 512 * c2:512 * (c2 + 1)],
                    op=ALU.add)

        def emit_proj(b, c2, jo):
            py = pmm.tile([128, 512], F32, name="py", tag=f"pq{jo % 4}")
            for k in range(NK):
                nc.tensor.matmul(py[:], PWT[k][:, 128 * jo:128 * (jo + 1)],
                                 OTc[k][c2][:], start=(k == 0), stop=(k == NK - 1))
            ysb = wp.tile([128, 512], F32, name="ysb", tag="ysb", bufs=2)
            nc.scalar.activation(ysb[:], py[:], AF.Identity, bias=pjb[:, jo:jo + 1])
            t0 = 1024 * b + 512 * c2
            nc.sync.dma_start(out=y_out[128 * jo:128 * (jo + 1), t0:t0 + 512],
                              in_=ysb[:])

        # b0 einsum2 (both chunks), then proj b0 c2=0;
        # e2 b1 c2=0 interleaves into proj b0 c2=1 (OTc rings free as proj b0
        # finishes reading each chunk), e2 b1 c2=1 into proj b1 c2=0.
        for c2 in range(2):
            for h in range(H):
                emit_e2(0, c2, h)
        # batch-1 dwconv split: PE keeps the dy=+-1 taps (psum + ACT bias
        # evac), DVE adds the dy=0 row (fits its slack during proj b0).
        dwc_compute(1, dysel=(-1, 1))
        for g in range(KVH):
            v3 = vTt[1][g][:].rearrange("p (y x) -> p y x", y=32)
            o3 = vd[1][g][:].rearrange("p (y x) -> p y x", y=32)
            for ti, (dy, dx) in enumerate(TAPS):
                if dy != 0:
                    continue
                y0, y1 = 0, 32
                x0, x1 = max(0, -dx), 32 - max(0, dx)
                tmp = wp.tile([96, N], BF16, name="tmp", tag="dtmp", bufs=1)
                t3 = tmp[:].rearrange("p (y x) -> p y x", y=32)
                nc.vector.tensor_scalar(
                    out=t3[:, y0:y1, x0:x1],
                    in0=v3[0:96, y0 + dy:y1 + dy, x0 + dx:x1 + dx],
                    scalar1=dwcw[:, g, ti:ti + 1], scalar2=None, op0=ALU.mult)
                nc.vector.tensor_tensor(
                    out=o3[:, y0:y1, x0:x1], in0=o3[:, y0:y1, x0:x1],
                    in1=t3[:, y0:y1, x0:x1], op=ALU.add)
        for jo in range(NK):
            emit_proj(0, 0, jo)
        e2q = [(1, 0, h) for h in range(H)]
        for jo in range(NK):
            emit_proj(0, 1, jo)
            for _ in range(2):
                if e2q:
                    emit_e2(*e2q.pop(0))
        while e2q:
            emit_e2(*e2q.pop(0))
        e2q = [(1, 1, h) for h in range(H)]
        for jo in range(NK):
            emit_proj(1, 0, jo)
            for _ in range(2):
                if e2q:
                    emit_e2(*e2q.pop(0))
        while e2q:
            emit_e2(*e2q.pop(0))
        for jo in range(NK - 1):
            emit_proj(1, 1, jo)
        # final tile split in half-columns to pipeline the tail evac/DMA
        for half in range(2):
            py = pmm.tile([128, 512], F32, name="pyh", tag=f"pq{half}")
            for k in range(NK):
                nc.tensor.matmul(py[:, 0:256],
                                 PWT[k][:, 128 * 8:DIM],
                                 OTc[k][1][:, 256 * half:256 * (half + 1)],
                                 start=(k == 0), stop=(k == NK - 1))
            ysb = wp.tile([128, 512], F32, name="ysb", tag="ysb", bufs=2)
            nc.scalar.activation(ysb[:, 0:256], py[:, 0:256], AF.Identity,
                                 bias=pjb[:, 8:9])
            t0 = 1024 + 512 + 256 * half
            nc.sync.dma_start(out=y_out[128 * 8:DIM, t0:t0 + 256],
                              in_=ysb[:, 0:256])

    nc.compile()
    return nc


_NC_CACHE = None


def _get_nc():
    global _NC_CACHE
    if _NC_CACHE is None:
        _NC_CACHE = _build_kernel()
    return _NC_CACHE


def _host_consts(wq_w, wq_b, wkv_w, wkv_b, dwc_w, dwc_b, proj_w, proj_b):
    wqT = np.ascontiguousarray(np.asarray(wq_w, np.float32).T).astype(_BF)
    wkvT = np.ascontiguousarray(np.asarray(wkv_w, np.float32).T).astype(_BF)
    pwT = np.ascontiguousarray(np.asarray(proj_w, np.float32).T).astype(_BF)
    wqb = np.ascontiguousarray(np.asarray(wq_b, np.float32).reshape(NK, 128).T)
    kvb_full = np.asarray(wkv_b, np.float32).reshape(1, 768)
    kvb_r = kvb_full[:, 0:384].astype(_BF)
    kvbbc = np.broadcast_to(kvb_full[:, 384:768], (128, 384)).astype(_BF)
    pjb = np.ascontiguousarray(np.asarray(proj_b, np.float32).reshape(NK, 128).T)
    dw = np.asarray(dwc_w, np.float32).reshape(KVH, 96, 9)
    dwcw = np.ascontiguousarray(dw.transpose(1, 0, 2))
    dwcb = np.ascontiguousarray(np.asarray(dwc_b, np.float32).reshape(KVH, 96).T)
    mk = np.zeros((128, NK, H), np.float32)
    for j in range(NK):
        for p in range(128):
            f = 128 * j + p
            mk[p, j, f // 96] = 1.0
    masks = mk.astype(_BF)
    dgv = np.zeros((96, KVH, 9, 96), np.float32)
    for d in range(96):
        dgv[d, :, :, d] = dw[:, d, :]
    diagp = dgv.astype(_BF)
    return dict(wqT=wqT, wkvT=wkvT, pwT=pwT, wqb=wqb, kvb=kvb_r, kvbbc=kvbbc,
                pjb=pjb, dwcw=dwcw, dwcb=dwcb, masks=masks, diagp=diagp)


def kernel(x, wq_w, wq_b, wkv_w, wkv_b, dwc_w, dwc_b, proj_w, proj_b,
           _want_results=False, **_unused):
    nc = _get_nc()
    consts = _host_consts(wq_w, wq_b, wkv_w, wkv_b, dwc_w, dwc_b, proj_w, proj_b)
    x = np.asarray(x, np.float32)
    in_maps = []
    for c in range(NCORES):
        m = dict(consts)
        m["xT"] = np.ascontiguousarray(
            x[BL * c:BL * (c + 1)].reshape(T, DIM).T).astype(_BF)
        in_maps.append(m)
    res = bass_utils.run_bass_kernel_spmd(nc, in_maps, core_ids=list(range(NCORES)))
    y = np.stack([np.ascontiguousarray(res.results[c]["y"].T).reshape(BL, N, DIM)
                  for c in range(NCORES)])
    y = y.reshape(B, N, DIM)
    if _want_results:
        return y, res
    return y


# revision 58
# speedup vs baseline: 1.0211x; 1.0045x over previous
"""Trainium2 Bass kernel for DiT focused-linear-attention block (nn_DiT_9259949490457).

Data-parallel over batch: 16 batches -> 8 NeuronCores, 2 batches/core, no collectives.
Host pre-transposes x -> xT (and y back), so the PE does only essential GEMM
columns: q-GEMM, kv-GEMM, einsum1/2, proj (feature-major, bias as per-partition
ACT bias), plus the depthwise 3x3 conv as clipped diagonal matmuls (center tap
first so psum pending-zero state stays uniform). Focus-norm row sums run as DVE
mul+reduce pairs (TensorTensorReduce breaks on HW); acc2 alternates ACT/DVE for
balance. Per-head q3 tiles come from a DRAM roundtrip (contiguous-row reads,
SWDGE-issued); q3 spills are ACT-issued so the SP/HWDGE queue stays clear for
x chunks. einsum2 evacuates via ACT copy + DVE piece-adds into 128-row-aligned
OT chunk tiles consumed by proj.
"""

import numpy as np
import ml_dtypes

import concourse.bacc as bacc
import concourse.mybir as mybir
import concourse.tile as tile
from concourse import bass_utils

F32 = mybir.dt.float32
BF16 = mybir.dt.bfloat16
ALU = mybir.AluOpType
AF = mybir.ActivationFunctionType
AX = mybir.AxisListType

NCORES = 8
B, N, DIM = 16, 1024, 1152
H, KVH, HD = 12, 4, 96
BL = B // NCORES          # 2 local batches
T = BL * N                # 2048 local tokens
NK = DIM // 128           # 9 feature K-tiles
TT = N // 128             # 8 token tiles per batch
C4 = T // 512             # 4 free-dim chunks of 512 over all local tokens
TAPS = [(dy, dx) for dy in (-1, 0, 1) for dx in (-1, 0, 1)]

_BF = ml_dtypes.bfloat16


def _spanp(b):
    if b % 128 == 0:
        return 128
    if b % 64 == 0:
        return 64
    return 32


def _head_pieces(h):
    """Split head h's 96 feature rows into pieces legal for partition-offset
    access both at the 128-aligned global row (r0) and the within-head row (rr).
    Returns [(j_tile, r0, rr, cnt)]."""
    out = []
    rr = 0
    while rr < 96:
        gr = 96 * h + rr
        j, r0 = divmod(gr, 128)
        cnt = min(96 - rr, 128 - r0, _spanp(r0), _spanp(rr))
        out.append((j, r0, rr, cnt))
        rr += cnt
    return out


def _build_kernel():
    nc = bacc.Bacc("TRN2", target_bir_lowering=False, debug=False,
                   enable_asserts=True, num_devices=NCORES)
    xT_in = nc.dram_tensor("xT", [DIM, T], BF16, kind="ExternalInput").ap()
    wqT_in = nc.dram_tensor("wqT", [DIM, DIM], BF16, kind="ExternalInput").ap()
    wkvT_in = nc.dram_tensor("wkvT", [DIM, 768], BF16, kind="ExternalInput").ap()
    pwT_in = nc.dram_tensor("pwT", [DIM, DIM], BF16, kind="ExternalInput").ap()
    wqb_in = nc.dram_tensor("wqb", [128, NK], F32, kind="ExternalInput").ap()
    kvb_in = nc.dram_tensor("kvb", [1, 384], BF16, kind="ExternalInput").ap()
    kvbbc_in = nc.dram_tensor("kvbbc", [128, 384], BF16, kind="ExternalInput").ap()
    pjb_in = nc.dram_tensor("pjb", [128, NK], F32, kind="ExternalInput").ap()
    dwcw_in = nc.dram_tensor("dwcw", [96, KVH, 9], F32, kind="ExternalInput").ap()
    dwcb_in = nc.dram_tensor("dwcb", [96, KVH], F32, kind="ExternalInput").ap()
    masks_in = nc.dram_tensor("masks", [128, NK, H], BF16, kind="ExternalInput").ap()
    diagp_in = nc.dram_tensor("diagp", [96, KVH, 9, 96], BF16, kind="ExternalInput").ap()
    y_out = nc.dram_tensor("y", [DIM, T], F32, kind="ExternalOutput").ap()

    from contextlib import ExitStack
    with tile.TileContext(nc) as tc, ExitStack() as stack:
        cpool = stack.enter_context(tc.tile_pool(name="const", bufs=1))
        dpool = stack.enter_context(tc.tile_pool(name="dram", bufs=1, space="DRAM"))
        wp = stack.enter_context(tc.tile_pool(name="work", bufs=1))
        pmm = stack.enter_context(tc.tile_pool(name="pmm", bufs=1, space="PSUM"))
        pa = stack.enter_context(tc.tile_pool(name="pa", bufs=2, space="PSUM"))

        # ---- consts (Pool/SWDGE path, parallel with HWDGE x loads below) ----
        WqT = [cpool.tile([128, DIM], BF16, name=f"WqT{k}") for k in range(NK)]
        WkvT = [cpool.tile([128, 768], BF16, name=f"WkvT{k}") for k in range(NK)]
        PWT = [cpool.tile([128, DIM], BF16, name=f"PWT{k}") for k in range(NK)]
        wqb = cpool.tile([128, NK], F32, name="wqb")
        kvb = cpool.tile([1, 384], BF16, name="kvb")
        kvbbc = cpool.tile([128, 384], BF16, name="kvbbc")
        pjb = cpool.tile([128, NK], F32, name="pjb")
        dwcw = cpool.tile([96, KVH, 9], F32, name="dwcw")
        dwcb = cpool.tile([96, KVH], F32, name="dwcb")
        masks = cpool.tile([128, NK, H], BF16, name="masks")
        diagP = cpool.tile([96, KVH, 9, 96], BF16, name="diagP")
        ones_r = cpool.tile([1, 128], BF16, name="ones_r")
        ones_c = cpool.tile([128, 1], BF16, name="ones_c")
        nc.vector.memset(ones_r[:], 1.0)
        nc.vector.memset(ones_c[:], 1.0)
        # dummy Sqrt up front so the one activation-table load that covers
        # Sqrt happens at t~0 instead of mid-kernel before the norms
        sqd = cpool.tile([1, 1], F32, name="sqd")
        nc.vector.memset(sqd[:], 1.0)
        nc.scalar.activation(sqd[:], sqd[:], AF.Sqrt)

        xT = [cpool.tile([128, T], BF16, name=f"xT{k}") for k in range(NK)]
        # SP/HWDGE: interleave x chunk-0 with WqT (both needed immediately),
        # then x c4=1, then WkvT (scheduler hoists K work into G1), then the
        # rest of x. Pool/SWDGE: wqb first, then late-needed consts.
        for half in range(2):
            for k in range(NK):
                nc.sync.dma_start(
                    out=xT[k][:, 1024 * half:1024 * (half + 1)],
                    in_=xT_in[128 * k:128 * (k + 1), 1024 * half:1024 * (half + 1)])
        for k in range(NK):
            nc.gpsimd.dma_start(out=WqT[k][:], in_=wqT_in[128 * k:128 * (k + 1), :])
        nc.gpsimd.dma_start(out=wqb[:], in_=wqb_in[:])
        for k in range(NK):
            nc.gpsimd.dma_start(out=WkvT[k][:], in_=wkvT_in[128 * k:128 * (k + 1), :])
        nc.gpsimd.dma_start(out=kvb[:], in_=kvb_in[:])
        nc.gpsimd.dma_start(out=kvbbc[:], in_=kvbbc_in[:])
        nc.gpsimd.dma_start(out=masks[:], in_=masks_in[:])
        nc.gpsimd.dma_start(out=diagP[:], in_=diagp_in[:])
        nc.gpsimd.dma_start(out=dwcw[:], in_=dwcw_in[:])
        nc.gpsimd.dma_start(out=dwcb[:], in_=dwcb_in[:])
        for k in range(NK):
            nc.gpsimd.dma_start(out=PWT[k][:], in_=pwT_in[128 * k:128 * (k + 1), :])
        nc.gpsimd.dma_start(out=pjb[:], in_=pjb_in[:])

        vpad = dpool.tile([BL, N, KVH, 128], BF16, name="vpad")
        q3d = dpool.tile([BL, DIM, N], BF16, name="q3d")

        # accs: col = (j, c4) for q, (g, t) for k
        acc1q = wp.tile([128, NK, C4], F32, name="acc1q", tag="acc1q")
        acc2q = wp.tile([128, NK, C4], F32, name="acc2q", tag="acc2q")
        acc1k = wp.tile([128, KVH, 2 * TT], F32, name="acc1k", tag="acc1k")
        acc2k = wp.tile([128, KVH, 2 * TT], F32, name="acc2k", tag="acc2k")

        q3h = [[wp.tile([96, N], BF16, name=f"q3h{b}_{h}", tag=f"q3h_{h}", bufs=1)
                for h in range(H)] for b in range(BL)]

        # ---------------- phase G1: q GEMM + focus(q) ----------------
        for c4 in range(C4):
            t0 = 512 * c4
            for jg in ((0, 1, 2), (3, 4, 5), (6, 7, 8)):
                def _g1psum(j):
                    m = j % 5
                    if m < 4:
                        return pmm.tile([128, 512], F32, name=f"pq{m}",
                                        tag=f"pq{m}")
                    return pa.tile([128, 512], F32, name="pqx",
                                   tag="pe2", bufs=3)
                pq = {j: _g1psum(j) for j in jg}
                for k in range(NK):
                    for j in jg:
                        nc.tensor.matmul(pq[j][:], WqT[k][:, 128 * j:128 * (j + 1)],
                                         xT[k][:, t0:t0 + 512],
                                         start=(k == 0), stop=(k == NK - 1))
                for j in jg:
                    u = wp.tile([128, 512], BF16, name="u", tag="u", bufs=2)
                    nc.scalar.activation(u[:], pq[j][:], AF.Relu, bias=wqb[:, j:j + 1])
                    u2 = wp.tile([128, 512], BF16, name="u2", tag="u2", bufs=2)
                    nc.vector.tensor_mul(u2[:], u[:], u[:])
                    nc.vector.tensor_reduce(out=acc1q[:, j, c4:c4 + 1], in_=u2[:],
                                            axis=AX.X, op=ALU.add)
                    q3s = wp.tile([128, 512], BF16, name="q3s", tag="q3s", bufs=3)
                    nc.vector.tensor_mul(q3s[:], u2[:], u[:])
                    junk = wp.tile([128, 512], BF16, name="junk", tag="junk",
                                   bufs=2)
                    if (c4 + j) % 2 == 0:
                        nc.scalar.activation(junk[:], q3s[:], AF.Square,
                                             accum_out=acc2q[:, j, c4:c4 + 1])
                    else:
                        nc.vector.tensor_mul(junk[:], q3s[:], q3s[:])
                        nc.vector.tensor_reduce(out=acc2q[:, j, c4:c4 + 1],
                                                in_=junk[:], axis=AX.X, op=ALU.add)
                    b = c4 // 2
                    nc.scalar.dma_start(
                        out=q3d[b, 128 * j:128 * (j + 1),
                                512 * (c4 % 2):512 * (c4 % 2 + 1)],
                        in_=q3s[:])
            if c4 % 2 == 1:
                # batch c4//2's q3d fully written: fetch per-head tiles now so
                # they are resident long before einsum2 needs them.
                bq = c4 // 2
                for h in range(H):
                    nc.gpsimd.dma_start(out=q3h[bq][h][:],
                                        in_=q3d[bq, 96 * h:96 * (h + 1), :])

        # ---------------- phase K/V + per-batch tail ----------------
        k3 = [wp.tile([128, 384], BF16, name=f"k3_{t}", tag=f"k3_{t}")
              for t in range(2 * TT)]
        vv = [wp.tile([128, 384], BF16, name=f"v_{t}", tag=f"v_{t}")
              for t in range(2 * TT)]
        kvp = [[wp.tile([96, 96], BF16, name=f"kvp{b}_{h}", tag=f"kvp_{h}", bufs=2)
                for h in range(H)] for b in range(BL)]
        vd = [[wp.tile([96, N], BF16, name=f"vd{b}_{g}", tag=f"vd_{g}", bufs=1)
               for g in range(KVH)] for b in range(BL)]
        gbs = []

        vTt = [[wp.tile([128, N], BF16, name=f"vT{b}_{g}", tag=f"vT_{g}", bufs=1)
                for g in range(KVH)] for b in range(BL)]

        def emit_vtrans(b):
            for g in range(KVH):
                nc.sync.dma_start(out=vTt[b][g][:], in_=vpad[b, :, g, :],
                                  transpose=True)

        def dwc_compute(b, dysel=(-1, 0, 1)):
            # all 9 taps as clipped diagonal matmuls accumulating in psum
            # halves; dwcb bias folded into the ACT evacuation.
            for g in range(KVH):
                v3 = vTt[b][g][:].rearrange("p (y x) -> p y x", y=32)
                pd = [pa.tile([96, 512], F32, name=f"pd{hf}", tag="pe2", bufs=3)
                      for hf in range(2)]
                p3 = [pd[hf][:].rearrange("p (y x) -> p y x", y=16)
                      for hf in range(2)]
                emitted = [False, False]
                last = [None, None]
                plan = []
                # center tap first per half: full coverage under start=True so
                # psum pending-zero state stays uniform for later partial taps
                taps_sorted = sorted(enumerate(TAPS),
                                     key=lambda e: (e[1] != (0, 0) and
                                                    e[1] != (-1, 0),))
                for ti, (dy, dx) in taps_sorted:
                    if dy not in dysel:
                        continue
                    x0, x1 = max(0, -dx), 32 - max(0, dx)
                    for hf in (0, 1):
                        y0 = max(16 * hf, -dy if dy < 0 else 0)
                        y1 = min(16 * hf + 16, 32 - max(0, dy))
                        if y1 > y0:
                            plan.append((ti, dy, dx, hf, y0, y1, x0, x1))
                            last[hf] = len(plan) - 1
                for pi, (ti, dy, dx, hf, y0, y1, x0, x1) in enumerate(plan):
                    nc.tensor.matmul(
                        p3[hf][:, y0 - 16 * hf:y1 - 16 * hf, x0:x1],
                        diagP[:, g, ti, :],
                        v3[0:96, y0 + dy:y1 + dy, x0 + dx:x1 + dx],
                        start=not emitted[hf], stop=(pi == last[hf]))
                    emitted[hf] = True
                for hf in range(2):
                    nc.scalar.activation(vd[b][g][:, 512 * hf:512 * (hf + 1)],
                                         pd[hf][:], AF.Identity,
                                         bias=dwcb[:, g:g + 1])

        for b in range(BL):
          # keep K off the scheduler's early-hoist list until WkvT has landed
          with tc.tile_wait_until(0.030, enable=(b == 0)):
            for t in range(TT * b, TT * (b + 1)):
                t0 = 128 * t
                pk = pmm.tile([128, 512], F32, name="pk", tag=f"pq{t % 4}")
                for k in range(NK):
                    nc.tensor.matmul(pk[:, 0:384], xT[k][:, t0:t0 + 128],
                                     WkvT[k][:, 0:384],
                                     start=(k == 0), stop=False)
                nc.tensor.matmul(pk[:, 0:384], ones_r[:], kvb[:],
                                 start=False, stop=True)
                uk = wp.tile([128, 384], BF16, name="uk", tag="uk", bufs=2)
                nc.scalar.activation(uk[:], pk[:, 0:384], AF.Relu)
                k2 = wp.tile([128, 384], BF16, name="k2", tag="k2", bufs=2)
                nc.vector.tensor_mul(k2[:], uk[:], uk[:])
                nc.vector.tensor_reduce(
                    out=acc1k[:, :, t], in_=k2[:].rearrange("p (g d) -> p g d", g=KVH),
                    axis=AX.X, op=ALU.add)
                nc.vector.tensor_mul(k3[t][:], k2[:], uk[:])
                junkk = wp.tile([128, 384], BF16, name="junkk", tag="junk", bufs=2)
                nc.vector.tensor_mul(junkk[:], k3[t][:], k3[t][:])
                nc.vector.tensor_reduce(
                    out=acc2k[:, :, t], in_=junkk[:].rearrange("p (g d) -> p g d", g=KVH),
                    axis=AX.X, op=ALU.add)
            for t in range(TT * b, TT * (b + 1)):
                t0 = 128 * t
                pv = pmm.tile([128, 512], F32, name="pv", tag=f"pq{t % 4}")
                for k in range(NK):
                    nc.tensor.matmul(pv[:, 0:384], xT[k][:, t0:t0 + 128],
                                     WkvT[k][:, 384:768],
                                     start=(k == 0), stop=(k == NK - 1))
                nc.vector.tensor_tensor(out=vv[t][:], in0=pv[:, 0:384],
                                        in1=kvbbc[:], op=ALU.add)
                nc.sync.dma_start(
                    out=vpad[b, 128 * (t - TT * b):128 * (t - TT * b + 1), :, 0:96],
                    in_=vv[t][:].rearrange("p (k d) -> p k d", k=KVH))
            emit_vtrans(b)
            if b == 1:
                dwc_compute(0)

            # ---- norms -> per-head scale gb (tiny) ----
            sq_rows = []
            for acc in (acc1q, acc2q):
                accs = wp.tile([128, NK], F32, name="accs", tag="accs", bufs=1)
                nc.vector.tensor_add(accs[:], acc[:, :, 2 * b], acc[:, :, 2 * b + 1])
                accsb = wp.tile([128, NK], BF16, name="accsb", tag="accsb", bufs=1)
                nc.vector.tensor_copy(accsb[:], accs[:])
                psn = pa.tile([1, H], F32, name="psn", tag="pa", bufs=1)
                for j in range(NK):
                    nc.tensor.matmul(psn[:], accsb[:, j:j + 1], masks[:, j, :],
                                     start=(j == 0), stop=(j == NK - 1))
                srow = wp.tile([1, H], F32, name="srow", tag="srow", bufs=2)
                nc.vector.tensor_copy(srow[:], psn[:])
                sq_rows.append(srow)
            sk_rows = []
            for acc in (acc1k, acc2k):
                accb = wp.tile([128, KVH * TT], BF16, name="accb", tag="accb", bufs=1)
                nc.vector.tensor_copy(accb[:], acc[:, :, TT * b:TT * (b + 1)])
                psk = pa.tile([1, KVH * TT], F32, name="psk", tag="pa", bufs=1)
                nc.tensor.matmul(psk[:], ones_c[:], accb[:], start=True, stop=True)
                krow = wp.tile([1, KVH * TT], F32, name="krow", tag="krow", bufs=1)
                nc.vector.tensor_copy(krow[:], psk[:])
                kred = wp.tile([1, KVH], F32, name="kred", tag="kred", bufs=2)
                nc.vector.tensor_reduce(kred[:],
                                        krow[:].rearrange("a (k t) -> a k t", k=KVH),
                                        axis=AX.X, op=ALU.add)
                sk_rows.append(kred)

            def _f_row(s1, s2, width, tagp):
                se = wp.tile([1, width], F32, name="se", tag=f"se{tagp}", bufs=1)
                nc.vector.tensor_scalar_add(se[:], s2[:], 1e-30)
                rc = wp.tile([1, width], F32, name="rc", tag=f"rc{tagp}", bufs=1)
                nc.vector.reciprocal(rc[:], se[:])
                rt = wp.tile([1, width], F32, name="rt", tag=f"rt{tagp}", bufs=1)
                nc.vector.tensor_mul(rt[:], s1[:], rc[:])
                fr = wp.tile([1, width], F32, name="fr", tag=f"fr{tagp}", bufs=1)
                nc.scalar.activation(fr[:], rt[:], AF.Sqrt)
                return fr

            fq = _f_row(sq_rows[0], sq_rows[1], H, "q")
            fk = _f_row(sk_rows[0], sk_rows[1], KVH, "k")
            fk12 = wp.tile([1, H], F32, name="fk12", tag="fk12", bufs=1)
            for g in range(3):
                nc.vector.tensor_copy(fk12[:, 4 * g:4 * (g + 1)], fk[:])
            grow = wp.tile([1, H], F32, name="grow", tag="grow", bufs=1)
            nc.vector.tensor_mul(grow[:], fq[:], fk12[:])
            gb = wp.tile([96, H], F32, name="gb", tag="gb", bufs=2)
            nc.gpsimd.partition_broadcast(gb[:], grow[:], channels=96)
            gbs.append(gb)

            # ---- einsum1 + scale ----
            for g in range(KVH):
                pk_t = pa.tile([96, 96], F32, name="pkvt", tag="pa", bufs=1)
                for i, t in enumerate(range(TT * b, TT * (b + 1))):
                    nc.tensor.matmul(pk_t[:], k3[t][:, 96 * g:96 * (g + 1)],
                                     vv[t][:, 96 * g:96 * (g + 1)],
                                     start=(i == 0), stop=(i == TT - 1))
                for h in range(g, H, KVH):
                    nc.vector.tensor_scalar(out=kvp[b][h][:], in0=pk_t[:],
                                            scalar1=gb[:, h:h + 1], scalar2=None,
                                            op0=ALU.mult)


        # ---------------- einsum2 + combine -> OTc, proj ----------------
        OTc = [[wp.tile([128, 512], BF16, name=f"OT_{j}_{c}", tag=f"OT_{j}_{c}",
                        bufs=1) for c in range(2)] for j in range(NK)]

        def emit_e2(b, c2, h):
            g = h % KVH
            pe2 = pa.tile([96, 512], F32, name="pe2", tag="pe2", bufs=3)
            nc.tensor.matmul(pe2[:], kvp[b][h][:],
                             q3h[b][h][:, 512 * c2:512 * (c2 + 1)],
                             start=True, stop=True)
            pac = wp.tile([96, 512], BF16, name="pac", tag="pac", bufs=2)
            nc.scalar.copy(pac[:], pe2[:])
            for (j, r0, rr, cnt) in _head_pieces(h):
                nc.vector.tensor_tensor(
                    out=OTc[j][c2][r0:r0 + cnt, :],
                    in0=pac[rr:rr + cnt, :],
                    in1=vd[b][g][rr:rr + cnt, 512 * c2:512 * (c2 + 1)],
                    op=ALU.add)

        def emit_proj(b, c2, jo):
            py = pmm.tile([128, 512], F32, name="py", tag=f"pq{jo % 4}")
            for k in range(NK):
                nc.tensor.matmul(py[:], PWT[k][:, 128 * jo:128 * (jo + 1)],
                                 OTc[k][c2][:], start=(k == 0), stop=(k == NK - 1))
            ysb = wp.tile([128, 512], F32, name="ysb", tag="ysb", bufs=2)
            nc.scalar.activation(ysb[:], py[:], AF.Identity, bias=pjb[:, jo:jo + 1])
            t0 = 1024 * b + 512 * c2
            nc.sync.dma_start(out=y_out[128 * jo:128 * (jo + 1), t0:t0 + 512],
                              in_=ysb[:])

        # b0 einsum2 (both chunks), then proj b0 c2=0;
        # e2 b1 c2=0 interleaves into proj b0 c2=1 (OTc rings free as proj b0
        # finishes reading each chunk), e2 b1 c2=1 into proj b1 c2=0.
        for c2 in range(2):
            for h in range(H):
                emit_e2(0, c2, h)
        # batch-1 dwconv split: PE keeps the dy=+-1 taps (psum + ACT bias
        # evac), DVE adds the dy=0 row (fits its slack during proj b0).
        dwc_compute(1, dysel=(-1, 1))
        for g in range(KVH):
            v3 = vTt[1][g][:].rearrange("p (y x) -> p y x", y=32)
            o3 = vd[1][g][:].rearrange("p (y x) -> p y x", y=32)
            for ti, (dy, dx) in enumerate(TAPS):
                if dy != 0:
                    continue
                y0, y1 = 0, 32
                x0, x1 = max(0, -dx), 32 - max(0, dx)
                tmp = wp.tile([96, N], BF16, name="tmp", tag="dtmp", bufs=1)
                t3 = tmp[:].rearrange("p (y x) -> p y x", y=32)
                nc.vector.tensor_scalar(
                    out=t3[:, y0:y1, x0:x1],
                    in0=v3[0:96, y0 + dy:y1 + dy, x0 + dx:x1 + dx],
                    scalar1=dwcw[:, g, ti:ti + 1], scalar2=None, op0=ALU.mult)
                nc.vector.tensor_tensor(
                    out=o3[:, y0:y1, x0:x1], in0=o3[:, y0:y1, x0:x1],
                    in1=t3[:, y0:y1, x0:x1], op=ALU.add)
        for jo in range(NK):
            emit_proj(0, 0, jo)
        e2q = [(1, 0, h) for h in range(H)]
        for jo in range(NK):
            emit_proj(0, 1, jo)
            for _ in range(2):
                if e2q:
                    emit_e2(*e2q.pop(0))
        while e2q:
            emit_e2(*e2q.pop(0))
        e2q = [(1, 1, h) for h in range(H)]
        for jo in range(NK):
            emit_proj(1, 0, jo)
            for _ in range(2):
                if e2q:
                    emit_e2(*e2q.pop(0))
        while e2q:
            emit_e2(*e2q.pop(0))
        for jo in range(NK):
            emit_proj(1, 1, jo)

    nc.compile()
    return nc


_NC_CACHE = None


def _get_nc():
    global _NC_CACHE
    if _NC_CACHE is None:
        _NC_CACHE = _build_kernel()
    return _NC_CACHE


def _host_consts(wq_w, wq_b, wkv_w, wkv_b, dwc_w, dwc_b, proj_w, proj_b):
    wqT = np.ascontiguousarray(np.asarray(wq_w, np.float32).T).astype(_BF)
    wkvT = np.ascontiguousarray(np.asarray(wkv_w, np.float32).T).astype(_BF)
    pwT = np.ascontiguousarray(np.asarray(proj_w, np.float32).T).astype(_BF)
    wqb = np.ascontiguousarray(np.asarray(wq_b, np.float32).reshape(NK, 128).T)
    kvb_full = np.asarray(wkv_b, np.float32).reshape(1, 768)
    kvb_r = kvb_full[:, 0:384].astype(_BF)
    kvbbc = np.broadcast_to(kvb_full[:, 384:768], (128, 384)).astype(_BF)
    pjb = np.ascontiguousarray(np.asarray(proj_b, np.float32).reshape(NK, 128).T)
    dw = np.asarray(dwc_w, np.float32).reshape(KVH, 96, 9)
    dwcw = np.ascontiguousarray(dw.transpose(1, 0, 2))
    dwcb = np.ascontiguousarray(np.asarray(dwc_b, np.float32).reshape(KVH, 96).T)
    mk = np.zeros((128, NK, H), np.float32)
    for j in range(NK):
        for p in range(128):
            f = 128 * j + p
            mk[p, j, f // 96] = 1.0
    masks = mk.astype(_BF)
    dgv = np.zeros((96, KVH, 9, 96), np.float32)
    for d in range(96):
        dgv[d, :, :, d] = dw[:, d, :]
    diagp = dgv.astype(_BF)
    return dict(wqT=wqT, wkvT=wkvT, pwT=pwT, wqb=wqb, kvb=kvb_r, kvbbc=kvbbc,
                pjb=pjb, dwcw=dwcw, dwcb=dwcb, masks=masks, diagp=diagp)


def kernel(x, wq_w, wq_b, wkv_w, wkv_b, dwc_w, dwc_b, proj_w, proj_b,
           _want_results=False, **_unused):
    nc = _get_nc()
    consts = _host_consts(wq_w, wq_b, wkv_w, wkv_b, dwc_w, dwc_b, proj_w, proj_b)
    x = np.asarray(x, np.float32)
    in_maps = []
    for c in range(NCORES):
        m = dict(consts)
        m["xT"] = np.ascontiguousarray(
            x[BL * c:BL * (c + 1)].reshape(T, DIM).T).astype(_BF)
        in_maps.append(m)
    res = bass_utils.run_bass_kernel_spmd(nc, in_maps, core_ids=list(range(NCORES)))
    y = np.stack([np.ascontiguousarray(res.results[c]["y"].T).reshape(BL, N, DIM)
                  for c in range(NCORES)])
    y = y.reshape(B, N, DIM)
    if _want_results:
        return y, res
    return y


# revision 59
# speedup vs baseline: 1.0237x; 1.0025x over previous
"""Trainium2 Bass kernel for DiT focused-linear-attention block (nn_DiT_9259949490457).

Data-parallel over batch: 16 batches -> 8 NeuronCores, 2 batches/core, no collectives.
Host pre-transposes x -> xT (and y back), so the PE does only essential GEMM
columns: q-GEMM, kv-GEMM, einsum1/2, proj (feature-major, bias as per-partition
ACT bias), plus the depthwise 3x3 conv as clipped diagonal matmuls (center tap
first so psum pending-zero state stays uniform). Focus-norm row sums run as DVE
mul+reduce pairs (TensorTensorReduce breaks on HW); acc2 alternates ACT/DVE for
balance. Per-head q3 tiles come from a DRAM roundtrip (contiguous-row reads,
SWDGE-issued); q3 spills are ACT-issued so the SP/HWDGE queue stays clear for
x chunks. einsum2 evacuates via ACT copy + DVE piece-adds into 128-row-aligned
OT chunk tiles consumed by proj.
"""

import numpy as np
import ml_dtypes

import concourse.bacc as bacc
import concourse.mybir as mybir
import concourse.tile as tile
from concourse import bass_utils

F32 = mybir.dt.float32
BF16 = mybir.dt.bfloat16
ALU = mybir.AluOpType
AF = mybir.ActivationFunctionType
AX = mybir.AxisListType

NCORES = 8
B, N, DIM = 16, 1024, 1152
H, KVH, HD = 12, 4, 96
BL = B // NCORES          # 2 local batches
T = BL * N                # 2048 local tokens
NK = DIM // 128           # 9 feature K-tiles
TT = N // 128             # 8 token tiles per batch
C4 = T // 512             # 4 free-dim chunks of 512 over all local tokens
TAPS = [(dy, dx) for dy in (-1, 0, 1) for dx in (-1, 0, 1)]

_BF = ml_dtypes.bfloat16


def _spanp(b):
    if b % 128 == 0:
        return 128
    if b % 64 == 0:
        return 64
    return 32


def _head_pieces(h):
    """Split head h's 96 feature rows into pieces legal for partition-offset
    access both at the 128-aligned global row (r0) and the within-head row (rr).
    Returns [(j_tile, r0, rr, cnt)]."""
    out = []
    rr = 0
    while rr < 96:
        gr = 96 * h + rr
        j, r0 = divmod(gr, 128)
        cnt = min(96 - rr, 128 - r0, _spanp(r0), _spanp(rr))
        out.append((j, r0, rr, cnt))
        rr += cnt
    return out


def _build_kernel():
    nc = bacc.Bacc("TRN2", target_bir_lowering=False, debug=False,
                   enable_asserts=True, num_devices=NCORES)
    xT_in = nc.dram_tensor("xT", [DIM, T], BF16, kind="ExternalInput").ap()
    wqT_in = nc.dram_tensor("wqT", [DIM, DIM], BF16, kind="ExternalInput").ap()
    wkvT_in = nc.dram_tensor("wkvT", [DIM, 768], BF16, kind="ExternalInput").ap()
    pwT_in = nc.dram_tensor("pwT", [DIM, DIM], BF16, kind="ExternalInput").ap()
    wqb_in = nc.dram_tensor("wqb", [128, NK], F32, kind="ExternalInput").ap()
    kvb_in = nc.dram_tensor("kvb", [1, 384], BF16, kind="ExternalInput").ap()
    kvbbc_in = nc.dram_tensor("kvbbc", [128, 384], BF16, kind="ExternalInput").ap()
    pjb_in = nc.dram_tensor("pjb", [128, NK], F32, kind="ExternalInput").ap()
    dwcw_in = nc.dram_tensor("dwcw", [96, KVH, 9], F32, kind="ExternalInput").ap()
    dwcb_in = nc.dram_tensor("dwcb", [96, KVH], F32, kind="ExternalInput").ap()
    masks_in = nc.dram_tensor("masks", [128, NK, H], BF16, kind="ExternalInput").ap()
    diagp_in = nc.dram_tensor("diagp", [96, KVH, 9, 96], BF16, kind="ExternalInput").ap()
    y_out = nc.dram_tensor("y", [DIM, T], F32, kind="ExternalOutput").ap()

    from contextlib import ExitStack
    with tile.TileContext(nc) as tc, ExitStack() as stack:
        cpool = stack.enter_context(tc.tile_pool(name="const", bufs=1))
        dpool = stack.enter_context(tc.tile_pool(name="dram", bufs=1, space="DRAM"))
        wp = stack.enter_context(tc.tile_pool(name="work", bufs=1))
        pmm = stack.enter_context(tc.tile_pool(name="pmm", bufs=1, space="PSUM"))
        pa = stack.enter_context(tc.tile_pool(name="pa", bufs=2, space="PSUM"))

        # ---- consts (Pool/SWDGE path, parallel with HWDGE x loads below) ----
        WqT = [cpool.tile([128, DIM], BF16, name=f"WqT{k}") for k in range(NK)]
        WkvT = [cpool.tile([128, 768], BF16, name=f"WkvT{k}") for k in range(NK)]
        PWT = [cpool.tile([128, DIM], BF16, name=f"PWT{k}") for k in range(NK)]
        wqb = cpool.tile([128, NK], F32, name="wqb")
        kvb = cpool.tile([1, 384], BF16, name="kvb")
        kvbbc = cpool.tile([128, 384], BF16, name="kvbbc")
        pjb = cpool.tile([128, NK], F32, name="pjb")
        dwcw = cpool.tile([96, KVH, 9], F32, name="dwcw")
        dwcb = cpool.tile([96, KVH], F32, name="dwcb")
        masks = cpool.tile([128, NK, H], BF16, name="masks")
        diagP = cpool.tile([96, KVH, 9, 96], BF16, name="diagP")
        ones_r = cpool.tile([1, 128], BF16, name="ones_r")
        ones_c = cpool.tile([128, 1], BF16, name="ones_c")
        nc.vector.memset(ones_r[:], 1.0)
        nc.vector.memset(ones_c[:], 1.0)
        # dummy Sqrt up front so the one activation-table load that covers
        # Sqrt happens at t~0 instead of mid-kernel before the norms
        sqd = cpool.tile([1, 1], F32, name="sqd")
        nc.vector.memset(sqd[:], 1.0)
        nc.scalar.activation(sqd[:], sqd[:], AF.Sqrt)

        xT = [cpool.tile([128, T], BF16, name=f"xT{k}") for k in range(NK)]
        # SP/HWDGE: interleave x chunk-0 with WqT (both needed immediately),
        # then x c4=1, then WkvT (scheduler hoists K work into G1), then the
        # rest of x. Pool/SWDGE: wqb first, then late-needed consts.
        for half in range(2):
            for k in range(NK):
                nc.sync.dma_start(
                    out=xT[k][:, 1024 * half:1024 * (half + 1)],
                    in_=xT_in[128 * k:128 * (k + 1), 1024 * half:1024 * (half + 1)])
        for k in range(NK):
            nc.gpsimd.dma_start(out=WqT[k][:], in_=wqT_in[128 * k:128 * (k + 1), :])
        nc.gpsimd.dma_start(out=wqb[:], in_=wqb_in[:])
        for k in range(NK):
            nc.gpsimd.dma_start(out=WkvT[k][:], in_=wkvT_in[128 * k:128 * (k + 1), :])
        nc.gpsimd.dma_start(out=kvb[:], in_=kvb_in[:])
        nc.gpsimd.dma_start(out=kvbbc[:], in_=kvbbc_in[:])
        nc.gpsimd.dma_start(out=masks[:], in_=masks_in[:])
        nc.gpsimd.dma_start(out=diagP[:], in_=diagp_in[:])
        nc.gpsimd.dma_start(out=dwcw[:], in_=dwcw_in[:])
        nc.gpsimd.dma_start(out=dwcb[:], in_=dwcb_in[:])
        for k in range(NK):
            nc.gpsimd.dma_start(out=PWT[k][:], in_=pwT_in[128 * k:128 * (k + 1), :])
        nc.gpsimd.dma_start(out=pjb[:], in_=pjb_in[:])

        vpad = dpool.tile([BL, N, KVH, 128], BF16, name="vpad")
        q3d = dpool.tile([BL, DIM, N], BF16, name="q3d")

        # accs: col = (j, c4) for q, (g, t) for k
        acc1q = wp.tile([128, NK, C4], F32, name="acc1q", tag="acc1q")
        acc2q = wp.tile([128, NK, C4], F32, name="acc2q", tag="acc2q")
        acc1k = wp.tile([128, KVH, 2 * TT], F32, name="acc1k", tag="acc1k")
        acc2k = wp.tile([128, KVH, 2 * TT], F32, name="acc2k", tag="acc2k")

        q3h = [[wp.tile([96, N], BF16, name=f"q3h{b}_{h}", tag=f"q3h_{h}", bufs=1)
                for h in range(H)] for b in range(BL)]

        # ---------------- phase G1: q GEMM + focus(q) ----------------
        for c4 in range(C4):
            t0 = 512 * c4
            for jg in ((0, 1, 2), (3, 4, 5), (6, 7, 8)):
                def _g1psum(j):
                    m = j % 5
                    if m < 4:
                        return pmm.tile([128, 512], F32, name=f"pq{m}",
                                        tag=f"pq{m}")
                    return pa.tile([128, 512], F32, name="pqx",
                                   tag="pe2", bufs=3)
                pq = {j: _g1psum(j) for j in jg}
                for k in range(NK):
                    for j in jg:
                        nc.tensor.matmul(pq[j][:], WqT[k][:, 128 * j:128 * (j + 1)],
                                         xT[k][:, t0:t0 + 512],
                                         start=(k == 0), stop=(k == NK - 1))
                for j in jg:
                    u = wp.tile([128, 512], BF16, name="u", tag="u", bufs=2)
                    nc.scalar.activation(u[:], pq[j][:], AF.Relu, bias=wqb[:, j:j + 1])
                    u2 = wp.tile([128, 512], BF16, name="u2", tag="u2", bufs=2)
                    q3s = wp.tile([128, 512], BF16, name="q3s", tag="q3s", bufs=3)
                    junk = wp.tile([128, 512], BF16, name="junk", tag="junk",
                                   bufs=2)
                    # balance: per tile exactly one Square+accum on ACT and
                    # one mul+reduce pair on DVE (else DVE paces the GEMM)
                    if (c4 + j) % 2 == 0:
                        nc.vector.tensor_mul(u2[:], u[:], u[:])
                        nc.vector.tensor_reduce(out=acc1q[:, j, c4:c4 + 1],
                                                in_=u2[:], axis=AX.X, op=ALU.add)
                        nc.vector.tensor_mul(q3s[:], u2[:], u[:])
                        nc.scalar.activation(junk[:], q3s[:], AF.Square,
                                             accum_out=acc2q[:, j, c4:c4 + 1])
                    else:
                        nc.scalar.activation(u2[:], u[:], AF.Square,
                                             accum_out=acc1q[:, j, c4:c4 + 1])
                        nc.vector.tensor_mul(q3s[:], u2[:], u[:])
                        nc.vector.tensor_mul(junk[:], q3s[:], q3s[:])
                        nc.vector.tensor_reduce(out=acc2q[:, j, c4:c4 + 1],
                                                in_=junk[:], axis=AX.X, op=ALU.add)
                    b = c4 // 2
                    nc.scalar.dma_start(
                        out=q3d[b, 128 * j:128 * (j + 1),
                                512 * (c4 % 2):512 * (c4 % 2 + 1)],
                        in_=q3s[:])
            if c4 % 2 == 1:
                # batch c4//2's q3d fully written: fetch per-head tiles now so
                # they are resident long before einsum2 needs them.
                bq = c4 // 2
                for h in range(H):
                    nc.gpsimd.dma_start(out=q3h[bq][h][:],
                                        in_=q3d[bq, 96 * h:96 * (h + 1), :])

        # ---------------- phase K/V + per-batch tail ----------------
        k3 = [wp.tile([128, 384], BF16, name=f"k3_{t}", tag=f"k3_{t}")
              for t in range(2 * TT)]
        vv = [wp.tile([128, 384], BF16, name=f"v_{t}", tag=f"v_{t}")
              for t in range(2 * TT)]
        kvp = [[wp.tile([96, 96], BF16, name=f"kvp{b}_{h}", tag=f"kvp_{h}", bufs=2)
                for h in range(H)] for b in range(BL)]
        vd = [[wp.tile([96, N], BF16, name=f"vd{b}_{g}", tag=f"vd_{g}", bufs=1)
               for g in range(KVH)] for b in range(BL)]
        gbs = []

        vTt = [[wp.tile([128, N], BF16, name=f"vT{b}_{g}", tag=f"vT_{g}", bufs=1)
                for g in range(KVH)] for b in range(BL)]

        def emit_vtrans(b):
            for g in range(KVH):
                nc.sync.dma_start(out=vTt[b][g][:], in_=vpad[b, :, g, :],
                                  transpose=True)

        def dwc_compute(b, dysel=(-1, 0, 1)):
            # all 9 taps as clipped diagonal matmuls accumulating in psum
            # halves; dwcb bias folded into the ACT evacuation.
            for g in range(KVH):
                v3 = vTt[b][g][:].rearrange("p (y x) -> p y x", y=32)
                pd = [pa.tile([96, 512], F32, name=f"pd{hf}", tag="pe2", bufs=3)
                      for hf in range(2)]
                p3 = [pd[hf][:].rearrange("p (y x) -> p y x", y=16)
                      for hf in range(2)]
                emitted = [False, False]
                last = [None, None]
                plan = []
                # center tap first per half: full coverage under start=True so
                # psum pending-zero state stays uniform for later partial taps
                taps_sorted = sorted(enumerate(TAPS),
                                     key=lambda e: (e[1] != (0, 0) and
                                                    e[1] != (-1, 0),))
                for ti, (dy, dx) in taps_sorted:
                    if dy not in dysel:
                        continue
                    x0, x1 = max(0, -dx), 32 - max(0, dx)
                    for hf in (0, 1):
                        y0 = max(16 * hf, -dy if dy < 0 else 0)
                        y1 = min(16 * hf + 16, 32 - max(0, dy))
                        if y1 > y0:
                            plan.append((ti, dy, dx, hf, y0, y1, x0, x1))
                            last[hf] = len(plan) - 1
                for pi, (ti, dy, dx, hf, y0, y1, x0, x1) in enumerate(plan):
                    nc.tensor.matmul(
                        p3[hf][:, y0 - 16 * hf:y1 - 16 * hf, x0:x1],
                        diagP[:, g, ti, :],
                        v3[0:96, y0 + dy:y1 + dy, x0 + dx:x1 + dx],
                        start=not emitted[hf], stop=(pi == last[hf]))
                    emitted[hf] = True
                for hf in range(2):
                    nc.scalar.activation(vd[b][g][:, 512 * hf:512 * (hf + 1)],
                                         pd[hf][:], AF.Identity,
                                         bias=dwcb[:, g:g + 1])

        for b in range(BL):
          # keep K off the scheduler's early-hoist list until WkvT has landed
          with tc.tile_wait_until(0.030, enable=(b == 0)):
            for t in range(TT * b, TT * (b + 1)):
                t0 = 128 * t
                pk = pmm.tile([128, 512], F32, name="pk", tag=f"pq{t % 4}")
                for k in range(NK):
                    nc.tensor.matmul(pk[:, 0:384], xT[k][:, t0:t0 + 128],
                                     WkvT[k][:, 0:384],
                                     start=(k == 0), stop=False)
                nc.tensor.matmul(pk[:, 0:384], ones_r[:], kvb[:],
                                 start=False, stop=True)
                uk = wp.tile([128, 384], BF16, name="uk", tag="uk", bufs=2)
                nc.scalar.activation(uk[:], pk[:, 0:384], AF.Relu)
                k2 = wp.tile([128, 384], BF16, name="k2", tag="k2", bufs=2)
                nc.scalar.activation(k2[:], uk[:], AF.Square)
                nc.vector.tensor_reduce(
                    out=acc1k[:, :, t], in_=k2[:].rearrange("p (g d) -> p g d", g=KVH),
                    axis=AX.X, op=ALU.add)
                nc.vector.tensor_mul(k3[t][:], k2[:], uk[:])
                junkk = wp.tile([128, 384], BF16, name="junkk", tag="junk", bufs=2)
                nc.vector.tensor_mul(junkk[:], k3[t][:], k3[t][:])
                nc.vector.tensor_reduce(
                    out=acc2k[:, :, t], in_=junkk[:].rearrange("p (g d) -> p g d", g=KVH),
                    axis=AX.X, op=ALU.add)
            for t in range(TT * b, TT * (b + 1)):
                t0 = 128 * t
                pv = pmm.tile([128, 512], F32, name="pv", tag=f"pq{t % 4}")
                for k in range(NK):
                    nc.tensor.matmul(pv[:, 0:384], xT[k][:, t0:t0 + 128],
                                     WkvT[k][:, 384:768],
                                     start=(k == 0), stop=(k == NK - 1))
                nc.vector.tensor_tensor(out=vv[t][:], in0=pv[:, 0:384],
                                        in1=kvbbc[:], op=ALU.add)
                nc.sync.dma_start(
                    out=vpad[b, 128 * (t - TT * b):128 * (t - TT * b + 1), :, 0:96],
                    in_=vv[t][:].rearrange("p (k d) -> p k d", k=KVH))
            emit_vtrans(b)
            if b == 1:
                dwc_compute(0)

            # ---- norms -> per-head scale gb (tiny) ----
            sq_rows = []
            for acc in (acc1q, acc2q):
                accs = wp.tile([128, NK], F32, name="accs", tag="accs", bufs=1)
                nc.vector.tensor_add(accs[:], acc[:, :, 2 * b], acc[:, :, 2 * b + 1])
                accsb = wp.tile([128, NK], BF16, name="accsb", tag="accsb", bufs=1)
                nc.vector.tensor_copy(accsb[:], accs[:])
                psn = pa.tile([1, H], F32, name="psn", tag="pa", bufs=1)
                for j in range(NK):
                    nc.tensor.matmul(psn[:], accsb[:, j:j + 1], masks[:, j, :],
                                     start=(j == 0), stop=(j == NK - 1))
                srow = wp.tile([1, H], F32, name="srow", tag="srow", bufs=2)
                nc.vector.tensor_copy(srow[:], psn[:])
                sq_rows.append(srow)
            sk_rows = []
            for acc in (acc1k, acc2k):
                accb = wp.tile([128, KVH * TT], BF16, name="accb", tag="accb", bufs=1)
                nc.vector.tensor_copy(accb[:], acc[:, :, TT * b:TT * (b + 1)])
                psk = pa.tile([1, KVH * TT], F32, name="psk", tag="pa", bufs=1)
                nc.tensor.matmul(psk[:], ones_c[:], accb[:], start=True, stop=True)
                krow = wp.tile([1, KVH * TT], F32, name="krow", tag="krow", bufs=1)
                nc.vector.tensor_copy(krow[:], psk[:])
                kred = wp.tile([1, KVH], F32, name="kred", tag="kred", bufs=2)
                nc.vector.tensor_reduce(kred[:],
                                        krow[:].rearrange("a (k t) -> a k t", k=KVH),
                                        axis=AX.X, op=ALU.add)
                sk_rows.append(kred)

            def _f_row(s1, s2, width, tagp):
                se = wp.tile([1, width], F32, name="se", tag=f"se{tagp}", bufs=1)
                nc.vector.tensor_scalar_add(se[:], s2[:], 1e-30)
                rc = wp.tile([1, width], F32, name="rc", tag=f"rc{tagp}", bufs=1)
                nc.vector.reciprocal(rc[:], se[:])
                rt = wp.tile([1, width], F32, name="rt", tag=f"rt{tagp}", bufs=1)
                nc.vector.tensor_mul(rt[:], s1[:], rc[:])
                fr = wp.tile([1, width], F32, name="fr", tag=f"fr{tagp}", bufs=1)
                nc.scalar.activation(fr[:], rt[:], AF.Sqrt)
                return fr

            fq = _f_row(sq_rows[0], sq_rows[1], H, "q")
            fk = _f_row(sk_rows[0], sk_rows[1], KVH, "k")
            fk12 = wp.tile([1, H], F32, name="fk12", tag="fk12", bufs=1)
            for g in range(3):
                nc.vector.tensor_copy(fk12[:, 4 * g:4 * (g + 1)], fk[:])
            grow = wp.tile([1, H], F32, name="grow", tag="grow", bufs=1)
            nc.vector.tensor_mul(grow[:], fq[:], fk12[:])
            gb = wp.tile([96, H], F32, name="gb", tag="gb", bufs=2)
            nc.gpsimd.partition_broadcast(gb[:], grow[:], channels=96)
            gbs.append(gb)

            # ---- einsum1 + scale ----
            for g in range(KVH):
                pk_t = pa.tile([96, 96], F32, name="pkvt", tag="pa", bufs=1)
                for i, t in enumerate(range(TT * b, TT * (b + 1))):
                    nc.tensor.matmul(pk_t[:], k3[t][:, 96 * g:96 * (g + 1)],
                                     vv[t][:, 96 * g:96 * (g + 1)],
                                     start=(i == 0), stop=(i == TT - 1))
                for h in range(g, H, KVH):
                    nc.vector.tensor_scalar(out=kvp[b][h][:], in0=pk_t[:],
                                            scalar1=gb[:, h:h + 1], scalar2=None,
                                            op0=ALU.mult)


        # ---------------- einsum2 + combine -> OTc, proj ----------------
        OTc = [[wp.tile([128, 512], BF16, name=f"OT_{j}_{c}", tag=f"OT_{j}_{c}",
                        bufs=1) for c in range(2)] for j in range(NK)]

        def emit_e2(b, c2, h):
            g = h % KVH
            pe2 = pa.tile([96, 512], F32, name="pe2", tag="pe2", bufs=3)
            nc.tensor.matmul(pe2[:], kvp[b][h][:],
                             q3h[b][h][:, 512 * c2:512 * (c2 + 1)],
                             start=True, stop=True)
            pac = wp.tile([96, 512], BF16, name="pac", tag="pac", bufs=2)
            nc.scalar.copy(pac[:], pe2[:])
            for (j, r0, rr, cnt) in _head_pieces(h):
                nc.vector.tensor_tensor(
                    out=OTc[j][c2][r0:r0 + cnt, :],
                    in0=pac[rr:rr + cnt, :],
                    in1=vd[b][g][rr:rr + cnt, 512 * c2:512 * (c2 + 1)],
                    op=ALU.add)

        def emit_proj(b, c2, jo):
            py = pmm.tile([128, 512], F32, name="py", tag=f"pq{jo % 4}")
            for k in range(NK):
                nc.tensor.matmul(py[:], PWT[k][:, 128 * jo:128 * (jo + 1)],
                                 OTc[k][c2][:], start=(k == 0), stop=(k == NK - 1))
            ysb = wp.tile([128, 512], F32, name="ysb", tag="ysb", bufs=2)
            nc.scalar.activation(ysb[:], py[:], AF.Identity, bias=pjb[:, jo:jo + 1])
            t0 = 1024 * b + 512 * c2
            nc.sync.dma_start(out=y_out[128 * jo:128 * (jo + 1), t0:t0 + 512],
                              in_=ysb[:])

        # b0 einsum2 (both chunks), then proj b0 c2=0;
        # e2 b1 c2=0 interleaves into proj b0 c2=1 (OTc rings free as proj b0
        # finishes reading each chunk), e2 b1 c2=1 into proj b1 c2=0.
        for c2 in range(2):
            for h in range(H):
                emit_e2(0, c2, h)
        # batch-1 dwconv split: PE keeps the dy=+-1 taps (psum + ACT bias
        # evac), DVE adds the dy=0 row (fits its slack during proj b0).
        dwc_compute(1, dysel=(-1, 1))
        for g in range(KVH):
            v3 = vTt[1][g][:].rearrange("p (y x) -> p y x", y=32)
            o3 = vd[1][g][:].rearrange("p (y x) -> p y x", y=32)
            for ti, (dy, dx) in enumerate(TAPS):
                if dy != 0:
                    continue
                y0, y1 = 0, 32
                x0, x1 = max(0, -dx), 32 - max(0, dx)
                tmp = wp.tile([96, N], BF16, name="tmp", tag="dtmp", bufs=1)
                t3 = tmp[:].rearrange("p (y x) -> p y x", y=32)
                nc.vector.tensor_scalar(
                    out=t3[:, y0:y1, x0:x1],
                    in0=v3[0:96, y0 + dy:y1 + dy, x0 + dx:x1 + dx],
                    scalar1=dwcw[:, g, ti:ti + 1], scalar2=None, op0=ALU.mult)
                nc.vector.tensor_tensor(
                    out=o3[:, y0:y1, x0:x1], in0=o3[:, y0:y1, x0:x1],
                    in1=t3[:, y0:y1, x0:x1], op=ALU.add)
        for jo in range(NK):
            emit_proj(0, 0, jo)
        e2q = [(1, 0, h) for h in range(H)]
        for jo in range(NK):
            emit_proj(0, 1, jo)
            for _ in range(2):
                if e2q:
                    emit_e2(*e2q.pop(0))
        while e2q:
            emit_e2(*e2q.pop(0))
        e2q = [(1, 1, h) for h in range(H)]
        for jo in range(NK):
            emit_proj(1, 0, jo)
            for _ in range(2):
                if e2q:
                    emit_e2(*e2q.pop(0))
        while e2q:
            emit_e2(*e2q.pop(0))
        for jo in range(NK):
            emit_proj(1, 1, jo)

    nc.compile()
    return nc


_NC_CACHE = None


def _get_nc():
    global _NC_CACHE
    if _NC_CACHE is None:
        _NC_CACHE = _build_kernel()
    return _NC_CACHE


def _host_consts(wq_w, wq_b, wkv_w, wkv_b, dwc_w, dwc_b, proj_w, proj_b):
    wqT = np.ascontiguousarray(np.asarray(wq_w, np.float32).T).astype(_BF)
    wkvT = np.ascontiguousarray(np.asarray(wkv_w, np.float32).T).astype(_BF)
    pwT = np.ascontiguousarray(np.asarray(proj_w, np.float32).T).astype(_BF)
    wqb = np.ascontiguousarray(np.asarray(wq_b, np.float32).reshape(NK, 128).T)
    kvb_full = np.asarray(wkv_b, np.float32).reshape(1, 768)
    kvb_r = kvb_full[:, 0:384].astype(_BF)
    kvbbc = np.broadcast_to(kvb_full[:, 384:768], (128, 384)).astype(_BF)
    pjb = np.ascontiguousarray(np.asarray(proj_b, np.float32).reshape(NK, 128).T)
    dw = np.asarray(dwc_w, np.float32).reshape(KVH, 96, 9)
    dwcw = np.ascontiguousarray(dw.transpose(1, 0, 2))
    dwcb = np.ascontiguousarray(np.asarray(dwc_b, np.float32).reshape(KVH, 96).T)
    mk = np.zeros((128, NK, H), np.float32)
    for j in range(NK):
        for p in range(128):
            f = 128 * j + p
            mk[p, j, f // 96] = 1.0
    masks = mk.astype(_BF)
    dgv = np.zeros((96, KVH, 9, 96), np.float32)
    for d in range(96):
        dgv[d, :, :, d] = dw[:, d, :]
    diagp = dgv.astype(_BF)
    return dict(wqT=wqT, wkvT=wkvT, pwT=pwT, wqb=wqb, kvb=kvb_r, kvbbc=kvbbc,
                pjb=pjb, dwcw=dwcw, dwcb=dwcb, masks=masks, diagp=diagp)


def kernel(x, wq_w, wq_b, wkv_w, wkv_b, dwc_w, dwc_b, proj_w, proj_b,
           _want_results=False, **_unused):
    nc = _get_nc()
    consts = _host_consts(wq_w, wq_b, wkv_w, wkv_b, dwc_w, dwc_b, proj_w, proj_b)
    x = np.asarray(x, np.float32)
    in_maps = []
    for c in range(NCORES):
        m = dict(consts)
        m["xT"] = np.ascontiguousarray(
            x[BL * c:BL * (c + 1)].reshape(T, DIM).T).astype(_BF)
        in_maps.append(m)
    res = bass_utils.run_bass_kernel_spmd(nc, in_maps, core_ids=list(range(NCORES)))
    y = np.stack([np.ascontiguousarray(res.results[c]["y"].T).reshape(BL, N, DIM)
                  for c in range(NCORES)])
    y = y.reshape(B, N, DIM)
    if _want_results:
        return y, res
    return y
